# revision 1
# baseline (speedup 1.0000x reference)
"""Trainium2 Bass kernel for GCN(x2) + MHA + mean + FC, sharded over 8 NeuronCores.

Sharding: 1D row partition of the 4096 nodes (512 rows/core). Each core holds
the column slice adj_hat[:, r*512:(r+1)*512] of the symmetric A+I (equal to its
row block transposed), all of x, and replicated weights. Cross-core exchanges
(on-device AllGather): degree vector, GCN1 output, K and V per head.

Optimizations over the v1 kernel:
- All large DMAs batched (host pre-packs adjacency/x/weights into SBUF-layout
  blocks), cutting DMA count ~7x.
- Adjacency + GCN inputs in fp8 (adjacency is binary -> exact) enabling
  DoubleRow matmuls (2 k-tiles per instruction) for degree/GCN contractions.
- exp() output in fp8 feeding DoubleRow den/ctx matmuls; softmax denominator
  via an fp8 ones-matmul instead of 64 vector adds.
- Attention query-sum folded: mean over nodes never materializes per-node
  attention output (out_proj applied to the summed context vector).
- Activation engine reserved for exp (activations/biases done on DVE/Pool) to
  avoid activation-table thrashing; exp reads [128,1024] PSUM blocks.
Host does only slicing/packing (shard) and an 8-way sum of [2]-vector partials.
"""
import sys
sys.path.insert(0, "/opt/trn_rl_repo")
import numpy as np
import ml_dtypes

N = 4096
NC_ = 8
R = N // NC_          # 512 rows per core
KB = N // 128         # 32 node chunks
F_IN = 128
G1 = 128
G2 = 512
HEADS = 4
HD = G2 // HEADS      # 128
ET = G2 // 128        # 4 tiles of the 512-dim embedding

# wf (f32 misc pack) column offsets
WF_B1 = 0      # [128,128] b1 broadcast
WF_B2 = 128    # [128,4]
WF_BQ = 132    # [128,4]
WF_BK = 136    # [128,4]
WF_BV = 140    # [128,4]
WF_BO = 144    # [128,4] bo/8
WF_FCW = 148   # [128,8]
WF_FCB = 156   # [1,2] fc_b/8 at partition 0
WF_COLS = 160

# wb (bf16 pack) column offsets
WB_W1 = 0          # [128,128]
WB_W2 = 128        # [128,512]
WB_QK = 640        # 4 c-tiles x [q 512 | k 512]
WB_WO = 640 + 4096 # 4 c-tiles x 512
WB_COLS = WB_WO + 2048

_cache = {}


def _build(sim1=False, rank=0):
    from concourse import bass, bacc, tile, mybir

    f32 = mybir.dt.float32
    bf16 = mybir.dt.bfloat16
    fp8 = mybir.dt.float8e4
    AF = mybir.ActivationFunctionType
    ALU = mybir.AluOpType
    PM = mybir.MatmulPerfMode
    AX = mybir.AxisListType
    from concourse.masks import make_identity

    nc = bacc.Bacc("TRN2", target_bir_lowering=False, debug=False,
                   num_devices=1 if sim1 else NC_)

    # ---- kernel I/O (per-core shards supplied via in_maps) ----
    adj_d = nc.dram_tensor("adjp", [128, KB, R], fp8, kind="ExternalInput")
    x_d = nc.dram_tensor("xp", [128, KB, F_IN], bf16, kind="ExternalInput")
    wb_d = nc.dram_tensor("wb", [128, WB_COLS], bf16, kind="ExternalInput")
    w8_d = nc.dram_tensor("w8", [128, ET, G2], fp8, kind="ExternalInput")
    wf_d = nc.dram_tensor("wf", [128, WF_COLS], f32, kind="ExternalInput")
    out_d = nc.dram_tensor("outp", [1, 2], f32, kind="ExternalOutput")

    dg_out = nc.dram_tensor("dg_out", [KB, 128], f32, kind="Internal",
                            addr_space="Shared")
    x1g = nc.dram_tensor("x1g", [NC_, 128, 4, G1], bf16, kind="Internal",
                         addr_space="Shared")
    kvo = nc.dram_tensor("kvo", [NC_, 128, HEADS, R], bf16, kind="Internal",
                         addr_space="Shared")
    vvo = nc.dram_tensor("vvo", [NC_, 4, 128, HEADS, HD], fp8, kind="Internal",
                         addr_space="Shared")

    RG = [list(range(NC_))]
    inv_sqrt_hd = 1.0 / float(np.sqrt(HD))

    with tile.TileContext(nc) as tc:
        with tc.tile_pool(name="wts", bufs=1) as wts, \
             tc.tile_pool(name="act", bufs=1) as actp, \
             tc.tile_pool(name="stg", bufs=2) as stg, \
             tc.tile_pool(name="ptp", bufs=2) as ptp, \
             tc.tile_pool(name="psc", bufs=2, space="PSUM") as psc, \
             tc.tile_pool(name="psd", bufs=2, space="PSUM") as psd, \
             tc.tile_pool(name="dram", bufs=1, space="DRAM") as drp:

            # ================= constants + weight/adj/x loads =================
            ones2 = wts.tile([128, 2, 128], fp8)
            nc.vector.memset(ones2[:, :, :], 1.0)
            ident = wts.tile([32, 32], f32)
            make_identity(nc, ident[:, :])

            adj8 = wts.tile([128, KB, R], fp8)
            for i in range(4):
                nc.sync.dma_start(adj8[:, 8 * i:8 * i + 8, :],
                                  adj_d[:, 8 * i:8 * i + 8, :])
            xbf = wts.tile([128, KB, F_IN], bf16)
            nc.sync.dma_start(xbf[:, :, :], x_d[:, :, :])
            wft = wts.tile([128, WF_COLS], f32)
            nc.sync.dma_start(wft[:], wf_d[:, :])
            wbt = wts.tile([128, WB_COLS], bf16)
            nc.sync.dma_start(wbt[:], wb_d[:, :])
            w8t = wts.tile([128, ET, G2], fp8)
            nc.sync.dma_start(w8t[:, :, :], w8_d[:, :, :])

            # ================= degree + AG1 =================
            ps_deg = psd.tile([128, 1024], f32, tag="d")
            for c in range(KB // 2):
                nc.tensor.matmul(ps_deg[:, 0:R], ones2[:, :, :],
                                 adj8[:, 2 * c:2 * c + 2, :],
                                 start=(c == 0), stop=(c == KB // 2 - 1),
                                 perf_mode=PM.DoubleRow)
            dsq = stg.tile([1, R], f32, tag="dsq")
            nc.scalar.activation(dsq[:], ps_deg[0:1, 0:R], AF.Sqrt)
            dloc = wts.tile([1, R], f32)
            nc.vector.reciprocal(dloc[:], dsq[:])
            dsqb = stg.tile([128, R], f32, tag="dsqb")
            nc.scalar.activation(dsqb[:], ps_deg[:, 0:R], AF.Sqrt)
            dbc = wts.tile([128, R], f32)
            nc.vector.reciprocal(dbc[:], dsqb[:])

            dg_in = drp.tile([1, R], f32, tag="dgin")
            nc.sync.dma_start(dg_in[:], dloc[:])
            if sim1:
                nc.sync.dma_start(dg_out[4*rank:4*rank+4, :].flatten(), dg_in[:].flatten())
            else:
                nc.gpsimd.collective_compute(
                    "AllGather", ALU.bypass, replica_groups=RG,
                    ins=[dg_in.opt()], outs=[dg_out.ap()])
            dg_sb = stg.tile([KB, 128], f32, tag="dgsb")
            nc.sync.dma_start(dg_sb[:, :], dg_out[:, :])
            ps_t = psd.tile([128, 1024], f32, tag="d")
            nc.tensor.transpose(ps_t[:, 0:KB], dg_sb[:, :], ident[:, :])
            dcol = wts.tile([128, KB], f32)
            nc.vector.tensor_copy(dcol[:], ps_t[:, 0:KB])

            # ================= GCN1 =================
            xs8 = actp.tile([128, KB, F_IN], fp8)
            for kb in range(KB):
                nc.vector.tensor_scalar_mul(xs8[:, kb, :], xbf[:, kb, :],
                                            dcol[:, kb:kb + 1])
            ps_s1 = psc.tile([128, 1024], f32, tag="c")
            for c in range(KB // 2):
                nc.tensor.matmul(ps_s1[:, 0:R], xs8[:, 2 * c:2 * c + 2, :],
                                 adj8[:, 2 * c:2 * c + 2, :],
                                 start=(c == 0), stop=(c == KB // 2 - 1),
                                 perf_mode=PM.DoubleRow)
            s1t = actp.tile([128, R], bf16)
            nc.vector.tensor_mul(s1t[:], ps_s1[:, 0:R], dbc[:])

            ps_x1 = psc.tile([128, 1024], f32, tag="c")
            for mt in range(4):
                nc.tensor.matmul(ps_x1[:, mt * G1:(mt + 1) * G1],
                                 s1t[:, mt * 128:(mt + 1) * 128],
                                 wbt[:, WB_W1:WB_W1 + G1],
                                 start=True, stop=True, skip_group_check=True)
            x1loc = actp.tile([128, 4, G1], bf16)
            for mt in range(4):
                x1tmp = stg.tile([128, G1], bf16, tag="x1tmp")
                nc.vector.tensor_add(x1tmp[:], ps_x1[:, mt * G1:(mt + 1) * G1],
                                     wft[:, WF_B1:WF_B1 + G1])
                nc.vector.tensor_scalar_max(x1loc[:, mt, :], x1tmp[:], 0.0)

            # AG2: gather x1 (bf16, [rank, p, mt, g])
            x1_in = drp.tile([128, 4, G1], bf16, tag="x1in")
            nc.sync.dma_start(x1_in[:, :, :], x1loc[:, :, :])
            if sim1:
                nc.sync.dma_start(x1g[rank:rank+1].flatten(), x1_in[:, :, :].flatten())
            else:
                nc.gpsimd.collective_compute(
                    "AllGather", ALU.bypass, replica_groups=RG,
                    ins=[x1_in.opt()], outs=[x1g.ap()])

            # ================= GCN2 =================
            x1st = actp.tile([128, NC_, 4, G1], bf16)
            for r in range(NC_):
                eng = nc.sync if r % 2 == 0 else nc.gpsimd
                eng.dma_start(x1st[:, r, :, :], x1g[r, :, :, :])
            x1s8 = actp.tile([128, KB, G1], fp8)
            for kb in range(KB):
                nc.vector.tensor_scalar_mul(x1s8[:, kb, :],
                                            x1st[:, kb // 4, kb % 4, :],
                                            dcol[:, kb:kb + 1])
            ps_s2 = psc.tile([128, 1024], f32, tag="c")
            for c in range(KB // 2):
                nc.tensor.matmul(ps_s2[:, 0:R], x1s8[:, 2 * c:2 * c + 2, :],
                                 adj8[:, 2 * c:2 * c + 2, :],
                                 start=(c == 0), stop=(c == KB // 2 - 1),
                                 perf_mode=PM.DoubleRow)
            s2t = actp.tile([128, R], bf16)
            nc.vector.tensor_mul(s2t[:], ps_s2[:, 0:R], dbc[:])

            # x2T tiles [e-tile 128, m 512] (bias per-partition)
            x2t = actp.tile([128, ET, R], bf16)
            x2t8 = actp.tile([128, ET, R], fp8)
            for eo in range(2):
                ps_x2 = psc.tile([128, 1024], f32, tag="c")
                for ei in range(2):
                    et = 2 * eo + ei
                    nc.tensor.matmul(ps_x2[:, ei * R:(ei + 1) * R],
                                     wbt[:, WB_W2 + et * 128:WB_W2 + (et + 1) * 128],
                                     s2t[:], start=True, stop=True,
                                     skip_group_check=True)
                for ei in range(2):
                    et = 2 * eo + ei
                    nc.vector.tensor_scalar_add(x2t[:, et, :],
                                                ps_x2[:, ei * R:(ei + 1) * R],
                                                wft[:, WF_B2 + et:WF_B2 + et + 1])
                    nc.vector.tensor_copy(x2t8[:, et, :], x2t[:, et, :])

            # ================= QKV + AG3 =================
            qt = actp.tile([128, HEADS, R], bf16)
            ktl = actp.tile([128, HEADS, R], bf16)
            vloc = actp.tile([128, 4, HEADS, HD], fp8)
            for h in range(HEADS):
                ps_qk = psc.tile([128, 1024], f32, tag="c")
                for c in range(ET):
                    qc = WB_QK + c * 1024 + h * 128
                    nc.tensor.matmul(ps_qk[:, 0:R], wbt[:, qc:qc + 128],
                                     x2t[:, c, :], start=(c == 0),
                                     stop=(c == ET - 1))
                for c in range(ET):
                    kc = WB_QK + c * 1024 + G2 + h * 128
                    nc.tensor.matmul(ps_qk[:, R:2 * R], wbt[:, kc:kc + 128],
                                     x2t[:, c, :], start=(c == 0),
                                     stop=(c == ET - 1))
                nc.vector.tensor_scalar_add(qt[:, h, :], ps_qk[:, 0:R],
                                            wft[:, WF_BQ + h:WF_BQ + h + 1])
                nc.vector.tensor_scalar_add(ktl[:, h, :], ps_qk[:, R:2 * R],
                                            wft[:, WF_BK + h:WF_BK + h + 1])

            # export K first so head-0 scores can start ASAP
            kvi = drp.tile([128, HEADS, R], bf16, tag="kvi")
            nc.sync.dma_start(kvi[:, :, :], ktl[:, :, :])
            if sim1:
                nc.sync.dma_start(kvo[rank:rank+1].flatten(), kvi[:, :, :].flatten())
            else:
                nc.gpsimd.collective_compute(
                    "AllGather", ALU.bypass, replica_groups=RG,
                    ins=[kvi.opt()], outs=[kvo.ap()])
            ktg = actp.tile([128, NC_, HEADS, R], bf16)
            for r in range(NC_):
                nc.sync.dma_start(ktg[:, r, :, :], kvo[r, :, :, :])

            for h in range(HEADS):
                ps_v = psd.tile([128, 1024], f32, tag="d")
                for mt in range(4):
                    for cp in range(2):
                        nc.tensor.matmul(
                            ps_v[:, mt * HD:(mt + 1) * HD],
                            x2t8[:, 2 * cp:2 * cp + 2, mt * 128:(mt + 1) * 128],
                            w8t[:, 2 * cp:2 * cp + 2, h * HD:(h + 1) * HD],
                            start=(cp == 0), stop=(cp == 1),
                            perf_mode=PM.DoubleRow, skip_group_check=True)
                for mt in range(4):
                    nc.vector.tensor_copy(vloc[:, mt, h, :],
                                          ps_v[:, mt * HD:(mt + 1) * HD])

            vvi = drp.tile([4, 128, HEADS, HD], fp8, tag="vvi")
            for mt in range(4):
                nc.gpsimd.dma_start(vvi[mt, :, :, :], vloc[:, mt, :, :])
            if sim1:
                nc.sync.dma_start(vvo[rank:rank+1].flatten(), vvi[:, :, :, :].flatten())
            else:
                nc.gpsimd.collective_compute(
                    "AllGather", ALU.bypass, replica_groups=RG,
                    ins=[vvi.opt()], outs=[vvo.ap()])
            vgl = actp.tile([128, HEADS, KB, HD], fp8)
            dmaeng = [nc.gpsimd, nc.gpsimd, nc.gpsimd, nc.sync]
            for r in range(NC_):
                for mt in range(4):
                    dmaeng[(r * 4 + mt) % 4].dma_start(
                        vgl[:, :, r * 4 + mt, :], vvo[r, mt, :, :, :])

            # ================= attention =================
            zb = wts.tile([128, HEADS], f32)
            nc.vector.tensor_scalar_mul(zb[:], wft[:, WF_BV:WF_BV + HEADS],
                                        float(R))
            zf = actp.tile([128, HEADS], f32)
            pts = {}
            cds = {}

            def pass2(hh):
                # denominator then context: contiguous DoubleRow groups
                ps_cd = psd.tile([128, 1024], f32, tag="d")
                cds[hh] = ps_cd
                pth = pts[hh]
                for pc in range(KB // 2):
                    nc.tensor.matmul(ps_cd[:, R:2 * R], ones2[:, :, :],
                                     pth[:, 2 * pc:2 * pc + 2, :],
                                     start=(pc == 0), stop=(pc == KB // 2 - 1),
                                     perf_mode=PM.DoubleRow,
                                     skip_group_check=True)
                for pc in range(KB // 2):
                    nc.tensor.matmul(ps_cd[:, 0:R],
                                     vgl[:, hh, 2 * pc:2 * pc + 2, :],
                                     pth[:, 2 * pc:2 * pc + 2, :],
                                     start=(pc == 0), stop=(pc == KB // 2 - 1),
                                     perf_mode=PM.DoubleRow,
                                     skip_group_check=True)

            def tail(hh):
                ps_cd = cds[hh]
                rbc = stg.tile([128, R], f32, tag="rbc")
                nc.vector.reciprocal(rbc[:], ps_cd[:, R:2 * R])
                ctxs = stg.tile([128, R], f32, tag="ctxs")
                nc.vector.tensor_mul(ctxs[:], ps_cd[:, 0:R], rbc[:])
                zr = stg.tile([128, 1], f32, tag="zr")
                nc.vector.tensor_reduce(zr[:], ctxs[:], axis=AX.X, op=ALU.add)
                nc.vector.tensor_add(zf[:, hh:hh + 1], zr[:], zb[:, hh:hh + 1])

            for h in range(HEADS):
                # pass 1: scores + exp into a per-head fp8 tile (DoubleRow
                # accumulation groups must not interleave with other matmuls)
                pt = ptp.tile([128, KB, R], fp8, tag="pt")
                pts[h] = pt
                for pc in range(KB // 2):
                    mc0, mc1 = 2 * pc, 2 * pc + 1
                    ps_sc = psc.tile([128, 1024], f32, tag="c")
                    nc.tensor.matmul(
                        ps_sc[:, 0:R],
                        ktg[:, mc0 // 4, h, (mc0 % 4) * 128:(mc0 % 4) * 128 + 128],
                        qt[:, h, :], start=True, stop=True,
                        skip_group_check=True)
                    nc.tensor.matmul(
                        ps_sc[:, R:2 * R],
                        ktg[:, mc1 // 4, h, (mc1 % 4) * 128:(mc1 % 4) * 128 + 128],
                        qt[:, h, :], start=True, stop=True,
                        skip_group_check=True)
                    nc.scalar.activation(pt[:, 2 * pc:2 * pc + 2, :],
                                         ps_sc[:, 0:2 * R], AF.Exp,
                                         scale=inv_sqrt_hd)
                if h >= 1:
                    pass2(h - 1)
                    tail(h - 1)
            pass2(HEADS - 1)
            tail(HEADS - 1)

            # ================= out_proj + mean + fc (partial) =================
            zb16 = actp.tile([128, HEADS], bf16)
            nc.vector.tensor_scalar_mul(zb16[:], zf[:], 1.0 / float(N))
            ps_u = psc.tile([128, 1024], f32, tag="c")
            for et in range(ET):
                for c in range(ET):
                    wc = WB_WO + c * G2 + et * 128
                    nc.tensor.matmul(ps_u[:, et:et + 1], wbt[:, wc:wc + 128],
                                     zb16[:, c:c + 1], start=(c == 0),
                                     stop=(c == ET - 1), skip_group_check=True)
            ub = actp.tile([128, ET], f32)
            for et in range(ET):
                nc.vector.tensor_scalar_add(ub[:, et:et + 1], ps_u[:, et:et + 1],
                                            wft[:, WF_BO + et:WF_BO + et + 1])
            ps_fc = psd.tile([128, 1024], f32, tag="d")
            for et in range(ET):
                nc.tensor.matmul(ps_fc[0:1, 0:2], ub[:, et:et + 1],
                                 wft[:, WF_FCW + 2 * et:WF_FCW + 2 * et + 2],
                                 start=(et == 0), stop=(et == ET - 1),
                                 skip_group_check=True)
            ores = stg.tile([1, 2], f32, tag="ores")
            nc.vector.tensor_add(ores[:], ps_fc[0:1, 0:2],
                                 wft[0:1, WF_FCB:WF_FCB + 2])
            nc.sync.dma_start(out_d[:, :], ores[:])

    nc.compile()
    return nc


def _pack_inputs(inputs):
    """Pack full inputs into per-core shards + replicated weight blocks."""
    fp8 = ml_dtypes.float8_e4m3
    bf16 = ml_dtypes.bfloat16

    adj = np.ascontiguousarray(inputs["adj_matrix"], dtype=np.float32)
    x = np.ascontiguousarray(inputs["node_features"], dtype=np.float32)
    W1 = np.asarray(inputs["W1"], np.float32)
    b1 = np.asarray(inputs["b1"], np.float32)
    W2 = np.asarray(inputs["W2"], np.float32)
    b2 = np.asarray(inputs["b2"], np.float32)
    ipw = np.asarray(inputs["in_proj_w"], np.float32)
    ipb = np.asarray(inputs["in_proj_b"], np.float32)
    wo = np.asarray(inputs["out_proj_w"], np.float32)
    bo = np.asarray(inputs["out_proj_b"], np.float32)
    fcw = np.asarray(inputs["fc_w"], np.float32)
    fcb = np.asarray(inputs["fc_b"], np.float32)

    wb = np.zeros((128, WB_COLS), np.float32)
    wb[:, WB_W1:WB_W1 + G1] = W1
    wb[:, WB_W2:WB_W2 + G2] = W2
    for c in range(ET):
        rows = slice(c * 128, (c + 1) * 128)
        wb[:, WB_QK + c * 1024:WB_QK + c * 1024 + G2] = ipw[rows, 0:G2]
        wb[:, WB_QK + c * 1024 + G2:WB_QK + (c + 1) * 1024] = ipw[rows, G2:2 * G2]
        wb[:, WB_WO + c * G2:WB_WO + (c + 1) * G2] = wo[rows, :]
    wb = wb.astype(bf16)

    w8 = np.zeros((128, ET, G2), np.float32)
    for c in range(ET):
        w8[:, c, :] = ipw[c * 128:(c + 1) * 128, 2 * G2:3 * G2]
    w8 = w8.astype(fp8)

    wf = np.zeros((128, WF_COLS), np.float32)
    wf[:, WF_B1:WF_B1 + G1] = np.broadcast_to(b1, (128, G1))
    wf[:, WF_B2:WF_B2 + ET] = b2.reshape(ET, 128).T
    wf[:, WF_BQ:WF_BQ + HEADS] = ipb[0:G2].reshape(HEADS, HD).T
    wf[:, WF_BK:WF_BK + HEADS] = ipb[G2:2 * G2].reshape(HEADS, HD).T
    wf[:, WF_BV:WF_BV + HEADS] = ipb[2 * G2:3 * G2].reshape(HEADS, HD).T
    wf[:, WF_BO:WF_BO + ET] = bo.reshape(ET, 128).T / NC_
    wf[:, WF_FCW:WF_FCW + 2 * ET] = fcw.reshape(ET, 128, 2).transpose(1, 0, 2) \
        .reshape(128, 2 * ET)
    wf[0, WF_FCB:WF_FCB + 2] = fcb / NC_

    xp = np.ascontiguousarray(
        x.reshape(KB, 128, F_IN).transpose(1, 0, 2)).astype(bf16)

    reps = {"wb": wb, "w8": w8, "wf": wf, "xp": xp}
    in_maps = []
    idx = np.arange(R)
    for r in range(NC_):
        cols = np.ascontiguousarray(adj[:, r * R:(r + 1) * R])
        cols[r * R + idx, idx] += 1.0   # A + I, this core's diagonal block
        adjp = np.ascontiguousarray(
            cols.reshape(KB, 128, R).transpose(1, 0, 2)).astype(fp8)
        in_maps.append({"adjp": adjp, **reps})
    return in_maps


def kernel(**inputs):
    from concourse.bass_utils import run_bass_kernel_spmd

    if "nc" not in _cache:
        _cache["nc"] = _build()
    nc = _cache["nc"]

    in_maps = _pack_inputs(inputs)
    res = run_bass_kernel_spmd(nc, in_maps, core_ids=list(range(NC_)))
    out = np.zeros(2, dtype=np.float64)
    for r in range(NC_):
        out += res.results[r]["outp"].reshape(2).astype(np.float64)
    return out.astype(np.float32)



# revision 11
# speedup vs baseline: 1.2516x; 1.2516x over previous
"""Trainium2 Bass kernel for GCN(x2) + MHA + mean + FC, sharded over 8 NeuronCores.

Sharding: 1D row partition of the 4096 nodes (512 rows/core). Each core holds
the column slice adj_hat[:, r*512:(r+1)*512] of the symmetric A+I (equal to its
row block transposed), all of x, and replicated weights. Cross-core exchanges
(on-device AllGather): degree vector, GCN1 output (dinv-prescaled fp8), K, V.

v2 changes over the 150us baseline:
- Softmax exp split across THREE engines: Act computes exp(s*ln2) straight
  from PSUM; DVE stages PSUM->SBUF copies; Pool computes 2^s via the pow ALU
  op from SBUF (Pool has no PSUM port). The 1/sqrt(hd)*log2(e) score scale is
  folded into qt at bias time, so probs = 2^score on every engine. This turns
  the attention phase from Act-bound (68us) into PE-bound (~41us).
- dinv = exp(-0.5*ln(deg)) on Act: ln/exp/relu/copy share ONE activation
  table, so the kernel performs a single table load, prewarmed at t=0.
- x shipped fp8; x1 AllGather carries dinv-prescaled fp8 (the GCN2-side
  rescale loop disappears; sender scale comes from a tiny transposed DMA of
  the core's own dinv, so the SPMD program stays rank-free); QKV projections
  run fp8 DoubleRow from x2t8.
- Adjacency DMA in 4 chunks with the degree matmul group chasing it; bulk
  weight loads are queued behind the degree roundtrip so the (single) DMA
  device is free when the latency-critical transfers arrive.
- K/V gathers staged per-rank with >=512B inner runs; no DMA issue on Pool.
Host does only slicing/packing (shard) and an 8-way sum of [2]-vector partials.
"""
import sys
sys.path.insert(0, "/opt/trn_rl_repo")
import numpy as np
import ml_dtypes

N = 4096
NC_ = 8
R = N // NC_          # 512 rows per core
KB = N // 128         # 32 node chunks
F_IN = 128
G1 = 128
G2 = 512
HEADS = 4
HD = G2 // HEADS      # 128
ET = G2 // 128        # 4 tiles of the 512-dim embedding

LN2 = float(np.log(2.0))
SCL = float(np.log2(np.e)) / float(np.sqrt(HD))  # probs = 2^(q.k) after fold

# wf (f32 misc pack) column offsets
WF_B1X4 = 0        # [128,512] b1 tiled 4x (broadcast over partitions)
WF_B2 = 512        # [128,4]
WF_BQ = 516        # [128,4]
WF_BK = 520        # [128,4]
WF_BV = 524        # [128,4]
WF_BO = 528        # [128,4] bo/8
WF_FCW = 532       # [128,8]
WF_FCB = 540       # [1,2] fc_b/8 at partition 0
WF_COLS = 542

# wb (bf16 pack) column offsets
WB_W1 = 0            # [128,128]
WB_W2 = 128          # [128,512]
WB_WO = 640          # 4 c-tiles x 512
WB_COLS = 640 + 2048

_cache = {}


def _build(sim1=False, rank=0):
    from concourse import bass, bacc, tile, mybir

    f32 = mybir.dt.float32
    bf16 = mybir.dt.bfloat16
    fp8 = mybir.dt.float8e4
    AF = mybir.ActivationFunctionType
    ALU = mybir.AluOpType
    PM = mybir.MatmulPerfMode
    AX = mybir.AxisListType

    nc = bacc.Bacc("TRN2", target_bir_lowering=False, debug=False,
                   num_devices=1 if sim1 else NC_)

    # ---- kernel I/O (per-core shards supplied via in_maps) ----
    adj_d = nc.dram_tensor("adjp", [128, KB, R], fp8, kind="ExternalInput")
    x_d = nc.dram_tensor("xp", [128, KB, F_IN], fp8, kind="ExternalInput")
    wb_d = nc.dram_tensor("wb", [128, WB_COLS], bf16, kind="ExternalInput")
    w8_d = nc.dram_tensor("w8", [128, ET, 3 * G2], fp8, kind="ExternalInput")
    wf_d = nc.dram_tensor("wf", [128, WF_COLS], f32, kind="ExternalInput")
    out_d = nc.dram_tensor("outp", [1, 2], f32, kind="ExternalOutput")

    dg_out = nc.dram_tensor("dg_out", [KB, 128], f32, kind="Internal",
                            addr_space="Shared")
    x1g = nc.dram_tensor("x1g", [NC_, 128, ET, G1], fp8, kind="Internal",
                         addr_space="Shared")
    kvo = nc.dram_tensor("kvo", [NC_, 128, HEADS, R], fp8, kind="Internal",
                         addr_space="Shared")
    vvo = nc.dram_tensor("vvo", [NC_, 128, HEADS, 4, HD], fp8, kind="Internal",
                         addr_space="Shared")

    RG = [list(range(NC_))]

    with tile.TileContext(nc) as tc:
        with tc.tile_pool(name="wts", bufs=1) as wts, \
             tc.tile_pool(name="act", bufs=1) as actp, \
             tc.tile_pool(name="stg", bufs=2) as stg, \
             tc.tile_pool(name="sce", bufs=3) as sce, \
             tc.tile_pool(name="ptp", bufs=2) as ptp, \
             tc.tile_pool(name="psc", bufs=4, space="PSUM") as psc, \
             tc.tile_pool(name="dram", bufs=1, space="DRAM") as drp:

            # ================= constants + act-table prewarm =================
            ones2 = wts.tile([128, 2, 128], fp8)
            nc.vector.memset(ones2[:, :, :], 1.0)
            two1k = wts.tile([128, 1024], f32)
            nc.gpsimd.memset(two1k[:, :], float(2.0 ** SCL))
            # prewarm the (ln,exp,relu,copy) table while the adjacency loads
            warm = wts.tile([1, 1], f32)
            nc.vector.memset(warm[:, :], 1.0)
            wsc = stg.tile([1, 1], f32, tag="warm")
            nc.scalar.activation(wsc[:, :], warm[:, :], AF.Sqrt)

            # ---- input loads. adj first (degree chases it); x on the scalar
            # queue; bulk weights queued on sync BEHIND the degree roundtrip
            # so the DMA device is free for the latency-critical hops.
            adj8 = wts.tile([128, KB, R], fp8)
            for i in range(4):
                nc.sync.dma_start(adj8[:, 8 * i:8 * i + 8, :],
                                  adj_d[:, 8 * i:8 * i + 8, :])
            wft = wts.tile([128, WF_COLS], f32)
            nc.sync.dma_start(wft[:], wf_d[:, :])
            x8 = wts.tile([128, KB, F_IN], fp8)
            for i in range(2):
                nc.scalar.dma_start(x8[:, 16 * i:16 * i + 16, :],
                                    x_d[:, 16 * i:16 * i + 16, :])

            # ================= degree (PE chases the adjacency DMA) ========
            ps_deg = psc.tile([128, 1024], f32, tag="c")
            for c in range(KB // 2):
                nc.tensor.matmul(ps_deg[:, 0:R], ones2[:, :, :],
                                 adj8[:, 2 * c:2 * c + 2, :],
                                 start=(c == 0), stop=(c == KB // 2 - 1),
                                 perf_mode=PM.DoubleRow)
            # dinv = 1/sqrt(deg); deg >= 1 always (A+I)
            dsq = stg.tile([128, R], f32, tag="dsq")
            nc.scalar.activation(dsq[:], ps_deg[:, 0:R], AF.Sqrt)
            dbc = wts.tile([128, R], f32)
            nc.vector.reciprocal(dbc[:], dsq[:])

            # ================= degree AllGather -> dcol =================
            dg_in = drp.tile([1, R], f32, tag="dgin")
            nc.sync.dma_start(dg_in[:], dbc[0:1, :])
            if sim1:
                nc.sync.dma_start(dg_out[4 * rank:4 * rank + 4, :].flatten(),
                                  dg_in[:].flatten())
            else:
                nc.gpsimd.collective_compute(
                    "AllGather", ALU.bypass, replica_groups=RG,
                    ins=[dg_in.opt()], outs=[dg_out.ap()])
            # dcol[p, kb] = dinv[kb*128+p] via transposed read of the gather
            dcol = wts.tile([128, KB], f32)
            nc.sync.dma_start(dcol[:, :], dg_out[:, :].transpose([1, 0]))
            # own dinv transposed to [node%128, mt] for the x1 sender scale
            down = wts.tile([128, ET], f32)
            nc.sync.dma_start(down[:, :],
                              dg_in[0, :].rearrange("(mt p) -> p mt", p=128))
            # bulk weights now (device free until x1 AG); W1/W2 first
            wbt = wts.tile([128, WB_COLS], bf16)
            nc.sync.dma_start(wbt[:, 0:WB_WO], wb_d[:, 0:WB_WO])
            w8t = wts.tile([128, ET, 3 * G2], fp8)
            nc.sync.dma_start(w8t[:, :, :], w8_d[:, :, :])
            nc.sync.dma_start(wbt[:, WB_WO:WB_COLS], wb_d[:, WB_WO:WB_COLS])

            # ================= GCN1 =================
            # xs8[:,kb,:] = x8[:,kb,:] * dinv[node]; 3-way engine split
            xs8 = actp.tile([128, KB, F_IN], fp8)
            for kb in range(KB):
                e = kb % 3
                if e == 0:
                    nc.vector.tensor_scalar_mul(xs8[:, kb, :], x8[:, kb, :],
                                                dcol[:, kb:kb + 1])
                elif e == 1:
                    nc.gpsimd.tensor_scalar_mul(xs8[:, kb, :], x8[:, kb, :],
                                                dcol[:, kb:kb + 1])
                else:
                    nc.scalar.activation(xs8[:, kb, :], x8[:, kb, :],
                                         AF.Copy, scale=dcol[:, kb:kb + 1])
            ps_s1 = psc.tile([128, 1024], f32, tag="c")
            for c in range(KB // 2):
                nc.tensor.matmul(ps_s1[:, 0:R], xs8[:, 2 * c:2 * c + 2, :],
                                 adj8[:, 2 * c:2 * c + 2, :],
                                 start=(c == 0), stop=(c == KB // 2 - 1),
                                 perf_mode=PM.DoubleRow)
            s1t = actp.tile([128, R], bf16)
            nc.vector.tensor_mul(s1t[:], ps_s1[:, 0:R], dbc[:])

            # W1 + bias, then relu with own-dinv prescale -> fp8 for the AG
            ps_x1 = psc.tile([128, 1024], f32, tag="c")
            for mt in range(4):
                nc.tensor.matmul(ps_x1[:, mt * G1:(mt + 1) * G1],
                                 s1t[:, mt * 128:(mt + 1) * 128],
                                 wbt[:, WB_W1:WB_W1 + G1],
                                 start=True, stop=True, skip_group_check=True)
            x1b = stg.tile([128, 4 * G1], f32, tag="x1b")
            nc.vector.tensor_add(x1b[:], ps_x1[:, 0:4 * G1],
                                 wft[:, WF_B1X4:WF_B1X4 + 4 * G1])
            x1s = actp.tile([128, ET, G1], fp8)
            for mt in range(4):
                # relu(x+b)*d == relu((x+b)*d) since d > 0
                nc.scalar.activation(x1s[:, mt, :],
                                     x1b[:, mt * G1:(mt + 1) * G1], AF.Relu,
                                     scale=down[:, mt:mt + 1])
            nc.scalar.activation(wsc[:, :], x1b[0:1, 0:1], AF.Exp)
            x1_in = drp.tile([128, ET, G1], fp8, tag="x1in")
            nc.sync.dma_start(x1_in[:, :, :], x1s[:, :, :])
            if sim1:
                nc.sync.dma_start(x1g[rank:rank + 1].flatten(),
                                  x1_in[:, :, :].flatten())
            else:
                nc.gpsimd.collective_compute(
                    "AllGather", ALU.bypass, replica_groups=RG,
                    ins=[x1_in.opt()], outs=[x1g.ap()])

            # ================= GCN2 =================
            x1a = actp.tile([128, NC_, ET, G1], fp8)
            if sim1:
                for r in range(NC_):
                    eng = [nc.sync, nc.scalar][r % 2]
                    eng.dma_start(x1a[:, r, :, :], x1g[rank, :, :, :])
            else:
                nc.sync.dma_start(x1a[:, :, :, :],
                                  x1g[:, :, :, :].transpose([1, 0, 2, 3]))
            ps_s2 = psc.tile([128, 1024], f32, tag="c")
            for c in range(KB // 2):
                r, mm = (2 * c) // 4, (2 * c) % 4
                nc.tensor.matmul(ps_s2[:, 0:R], x1a[:, r, mm:mm + 2, :],
                                 adj8[:, 2 * c:2 * c + 2, :],
                                 start=(c == 0), stop=(c == KB // 2 - 1),
                                 perf_mode=PM.DoubleRow)
            s2t = actp.tile([128, R], bf16)
            nc.vector.tensor_mul(s2t[:], ps_s2[:, 0:R], dbc[:])

            # W2: x2t8[e-tile, node] fp8 (+bias), feeds QKV via DoubleRow
            x2t8 = actp.tile([128, ET, R], fp8)
            for eo in range(2):
                ps_x2 = psc.tile([128, 1024], f32, tag="c")
                for ei in range(2):
                    et = 2 * eo + ei
                    nc.tensor.matmul(ps_x2[:, ei * R:(ei + 1) * R],
                                     wbt[:, WB_W2 + et * 128:WB_W2 + (et + 1) * 128],
                                     s2t[:], start=True, stop=True,
                                     skip_group_check=True)
                for ei in range(2):
                    et = 2 * eo + ei
                    if ei == 0:
                        nc.vector.tensor_scalar_add(
                            x2t8[:, et, :], ps_x2[:, ei * R:(ei + 1) * R],
                            wft[:, WF_B2 + et:WF_B2 + et + 1])
                    else:
                        nc.scalar.activation(
                            x2t8[:, et, :], ps_x2[:, ei * R:(ei + 1) * R],
                            AF.Identity,
                            bias=wft[:, WF_B2 + et:WF_B2 + et + 1])

            # ================= QKV (fp8 DoubleRow) =================
            qt = actp.tile([128, HEADS, R], fp8)
            ktl = actp.tile([128, HEADS, R], fp8)
            for hp in range(2):
                # K heads pair-wise first so the K AllGather starts early
                ps_qk = psc.tile([128, 1024], f32, tag="c")
                for hh in range(2):
                    h = 2 * hp + hh
                    for cp in range(2):
                        nc.tensor.matmul(
                            ps_qk[:, hh * R:(hh + 1) * R],
                            w8t[:, 2 * cp:2 * cp + 2, G2 + h * 128:G2 + (h + 1) * 128],
                            x2t8[:, 2 * cp:2 * cp + 2, :],
                            start=(cp == 0), stop=(cp == 1),
                            perf_mode=PM.DoubleRow, skip_group_check=True)
                for hh in range(2):
                    h = 2 * hp + hh
                    if hh == 0:
                        nc.vector.tensor_scalar_add(
                            ktl[:, h, :], ps_qk[:, hh * R:(hh + 1) * R],
                            wft[:, WF_BK + h:WF_BK + h + 1])
                    else:
                        nc.scalar.activation(
                            ktl[:, h, :], ps_qk[:, hh * R:(hh + 1) * R],
                            AF.Identity,
                            bias=wft[:, WF_BK + h:WF_BK + h + 1])

            # export K immediately; Q + V compute during the AG
            kvi = drp.tile([128, HEADS, R], fp8, tag="kvi")
            nc.sync.dma_start(kvi[:, :, :], ktl[:, :, :])
            if sim1:
                nc.sync.dma_start(kvo[rank:rank + 1].flatten(),
                                  kvi[:, :, :].flatten())
            else:
                nc.gpsimd.collective_compute(
                    "AllGather", ALU.bypass, replica_groups=RG,
                    ins=[kvi.opt()], outs=[kvo.ap()])

            for hp in range(2):
                ps_qk = psc.tile([128, 1024], f32, tag="c")
                for hh in range(2):
                    h = 2 * hp + hh
                    for cp in range(2):
                        nc.tensor.matmul(
                            ps_qk[:, hh * R:(hh + 1) * R],
                            w8t[:, 2 * cp:2 * cp + 2, h * 128:(h + 1) * 128],
                            x2t8[:, 2 * cp:2 * cp + 2, :],
                            start=(cp == 0), stop=(cp == 1),
                            perf_mode=PM.DoubleRow, skip_group_check=True)
                for hh in range(2):
                    h = 2 * hp + hh
                    # plain q + bq; the 1/sqrt(hd) softmax scale is folded
                    # into the exp bases (act scale / pool pow base)
                    if hh == 0:
                        nc.vector.tensor_scalar_add(
                            qt[:, h, :], ps_qk[:, hh * R:(hh + 1) * R],
                            wft[:, WF_BQ + h:WF_BQ + h + 1])
                    else:
                        nc.scalar.activation(
                            qt[:, h, :], ps_qk[:, hh * R:(hh + 1) * R],
                            AF.Identity,
                            bias=wft[:, WF_BQ + h:WF_BQ + h + 1])

            vloc = actp.tile([128, HEADS, 4 * HD], fp8)
            for h in range(HEADS):
                ps_v = psc.tile([128, 1024], f32, tag="c")
                for mt in range(4):
                    for cp in range(2):
                        nc.tensor.matmul(
                            ps_v[:, mt * HD:(mt + 1) * HD],
                            x2t8[:, 2 * cp:2 * cp + 2, mt * 128:(mt + 1) * 128],
                            w8t[:, 2 * cp:2 * cp + 2,
                                2 * G2 + h * HD:2 * G2 + (h + 1) * HD],
                            start=(cp == 0), stop=(cp == 1),
                            perf_mode=PM.DoubleRow, skip_group_check=True)
                if h % 2 == 0:
                    nc.vector.tensor_copy(vloc[:, h, :], ps_v[:, 0:4 * HD])
                else:
                    nc.scalar.activation(vloc[:, h, :], ps_v[:, 0:4 * HD],
                                         AF.Copy)
            vvi = drp.tile([128, HEADS, 4 * HD], fp8, tag="vvi")
            nc.sync.dma_start(vvi[:, :, :], vloc[:, :, :])
            if sim1:
                nc.sync.dma_start(vvo[rank:rank + 1].flatten(),
                                  vvi[:, :, :].flatten())
            else:
                nc.gpsimd.collective_compute(
                    "AllGather", ALU.bypass, replica_groups=RG,
                    ins=[vvi.opt()], outs=[vvo.ap()])

            # stage gathered K/V into SBUF, one DMA per rank (inner runs big).
            # K pieces first: scores(h0) chases them.
            ktg = actp.tile([128, NC_, HEADS, R], fp8)
            vgl = actp.tile([128, NC_, HEADS, 4, HD], fp8)
            for r in range(NC_):
                src_r = rank if sim1 else r
                eng = [nc.sync, nc.scalar][r % 2]
                eng.dma_start(ktg[:, r, :, :], kvo[src_r, :, :, :])
            for r in range(NC_):
                src_r = rank if sim1 else r
                eng = [nc.sync, nc.scalar][r % 2]
                eng.dma_start(vgl[:, r, :, :, :], vvo[src_r, :, :, :, :])

            # ================= attention =================
            zb = wts.tile([128, HEADS], f32)
            nc.vector.tensor_scalar_mul(zb[:], wft[:, WF_BV:WF_BV + HEADS],
                                        float(R))
            zf = actp.tile([128, HEADS], f32)
            junk = wts.tile([128, R], f32)
            pts = {}
            cds = {}

            def pass2(hh):
                # denominator then context: contiguous DoubleRow groups
                ps_cd = psc.tile([128, 1024], f32, tag="c")
                cds[hh] = ps_cd
                pth = pts[hh]
                for pc in range(KB // 2):
                    nc.tensor.matmul(ps_cd[:, R:2 * R], ones2[:, :, :],
                                     pth[:, 2 * pc:2 * pc + 2, :],
                                     start=(pc == 0), stop=(pc == KB // 2 - 1),
                                     perf_mode=PM.DoubleRow,
                                     skip_group_check=True)
                for pc in range(KB // 2):
                    r, mm = (2 * pc) // 4, (2 * pc) % 4
                    nc.tensor.matmul(ps_cd[:, 0:R],
                                     vgl[:, r, hh, mm:mm + 2, :],
                                     pth[:, 2 * pc:2 * pc + 2, :],
                                     start=(pc == 0), stop=(pc == KB // 2 - 1),
                                     perf_mode=PM.DoubleRow,
                                     skip_group_check=True)

            def tail(hh):
                ps_cd = cds[hh]
                rbc = stg.tile([128, R], f32, tag="rbc")
                nc.vector.reciprocal(rbc[:], ps_cd[:, R:2 * R])
                ctxs = stg.tile([128, R], f32, tag="ctxs")
                nc.vector.tensor_mul(ctxs[:], ps_cd[:, 0:R], rbc[:])
                zr = stg.tile([128, 1], f32, tag="zr")
                nc.gpsimd.tensor_reduce(zr[:], ctxs[:], axis=AX.X, op=ALU.add)
                nc.gpsimd.tensor_add(zf[:, hh:hh + 1], zr[:], zb[:, hh:hh + 1])

            # exp chunk routing: per head 16 chunks of [128,1024];
            # 9 -> Act (exp from PSUM), 7 -> DVE copy + Pool 2^x
            POOL_SETS = [{2, 5, 7, 10, 12, 15}, {1, 3, 5, 8, 10, 12, 14}]

            for h in range(HEADS):
                pt = ptp.tile([128, KB, R], fp8, tag="pt")
                pts[h] = pt
                for pc in range(KB // 2):
                    mc0, mc1 = 2 * pc, 2 * pc + 1
                    ps_sc = psc.tile([128, 1024], f32, tag="c")
                    nc.tensor.matmul(
                        ps_sc[:, 0:R],
                        ktg[:, mc0 // 4, h,
                            (mc0 % 4) * 128:(mc0 % 4) * 128 + 128],
                        qt[:, h, :], start=True, stop=True,
                        skip_group_check=True)
                    nc.tensor.matmul(
                        ps_sc[:, R:2 * R],
                        ktg[:, mc1 // 4, h,
                            (mc1 % 4) * 128:(mc1 % 4) * 128 + 128],
                        qt[:, h, :], start=True, stop=True,
                        skip_group_check=True)
                    if pc in POOL_SETS[h % 2]:
                        st = sce.tile([128, 1024], f32, tag="st")
                        nc.vector.tensor_copy(st[:, :], ps_sc[:, 0:1024])
                        nc.gpsimd.tensor_tensor(pt[:, 2 * pc:2 * pc + 2, :],
                                                two1k[:, :], st[:, :], ALU.pow)
                    else:
                        nc.scalar.activation(pt[:, 2 * pc:2 * pc + 2, :],
                                             ps_sc[:, 0:1024], AF.Exp,
                                             scale=LN2 * SCL)
                if h >= 1:
                    pass2(h - 1)
                    tail(h - 1)
            pass2(HEADS - 1)
            tail(HEADS - 1)

            # ================= out_proj + mean + fc (partial) ================
            zb16 = actp.tile([128, HEADS], bf16)
            nc.vector.tensor_scalar_mul(zb16[:], zf[:], 1.0 / float(N))
            ps_u = psc.tile([128, 1024], f32, tag="c")
            for et in range(ET):
                for c in range(ET):
                    wc = WB_WO + c * G2 + et * 128
                    nc.tensor.matmul(ps_u[:, et:et + 1], wbt[:, wc:wc + 128],
                                     zb16[:, c:c + 1], start=(c == 0),
                                     stop=(c == ET - 1), skip_group_check=True)
            ub = actp.tile([128, ET], f32)
            nc.vector.tensor_add(ub[:, :], ps_u[:, 0:ET],
                                 wft[:, WF_BO:WF_BO + ET])
            ps_fc = psc.tile([128, 1024], f32, tag="c")
            for et in range(ET):
                nc.tensor.matmul(ps_fc[0:1, 0:2], ub[:, et:et + 1],
                                 wft[:, WF_FCW + 2 * et:WF_FCW + 2 * et + 2],
                                 start=(et == 0), stop=(et == ET - 1),
                                 skip_group_check=True)
            ores = stg.tile([1, 2], f32, tag="ores")
            nc.vector.tensor_add(ores[:], ps_fc[0:1, 0:2],
                                 wft[0:1, WF_FCB:WF_FCB + 2])
            nc.sync.dma_start(out_d[:, :], ores[:])

    nc.compile()
    return nc


def _pack_inputs(inputs):
    """Pack full inputs into per-core shards + replicated weight blocks."""
    fp8 = ml_dtypes.float8_e4m3
    bf16 = ml_dtypes.bfloat16

    adj = np.ascontiguousarray(inputs["adj_matrix"], dtype=np.float32)
    x = np.ascontiguousarray(inputs["node_features"], dtype=np.float32)
    W1 = np.asarray(inputs["W1"], np.float32)
    b1 = np.asarray(inputs["b1"], np.float32)
    W2 = np.asarray(inputs["W2"], np.float32)
    b2 = np.asarray(inputs["b2"], np.float32)
    ipw = np.asarray(inputs["in_proj_w"], np.float32)
    ipb = np.asarray(inputs["in_proj_b"], np.float32)
    wo = np.asarray(inputs["out_proj_w"], np.float32)
    bo = np.asarray(inputs["out_proj_b"], np.float32)
    fcw = np.asarray(inputs["fc_w"], np.float32)
    fcb = np.asarray(inputs["fc_b"], np.float32)

    wb = np.zeros((128, WB_COLS), np.float32)
    wb[:, WB_W1:WB_W1 + G1] = W1
    wb[:, WB_W2:WB_W2 + G2] = W2
    for c in range(ET):
        rows = slice(c * 128, (c + 1) * 128)
        wb[:, WB_WO + c * G2:WB_WO + (c + 1) * G2] = wo[rows, :]
    wb = wb.astype(bf16)

    # fp8 QKV weights: [128, c-tile, 3*G2] (q | k | v)
    w8 = np.zeros((128, ET, 3 * G2), np.float32)
    for c in range(ET):
        w8[:, c, :] = ipw[c * 128:(c + 1) * 128, :]
    w8 = w8.astype(fp8)

    wf = np.zeros((128, WF_COLS), np.float32)
    wf[:, WF_B1X4:WF_B1X4 + 4 * G1] = np.tile(b1, 4)[None, :]
    wf[:, WF_B2:WF_B2 + ET] = b2.reshape(ET, 128).T
    wf[:, WF_BQ:WF_BQ + HEADS] = ipb[0:G2].reshape(HEADS, HD).T
    wf[:, WF_BK:WF_BK + HEADS] = ipb[G2:2 * G2].reshape(HEADS, HD).T
    wf[:, WF_BV:WF_BV + HEADS] = ipb[2 * G2:3 * G2].reshape(HEADS, HD).T
    wf[:, WF_BO:WF_BO + ET] = bo.reshape(ET, 128).T / NC_
    wf[:, WF_FCW:WF_FCW + 2 * ET] = fcw.reshape(ET, 128, 2).transpose(1, 0, 2) \
        .reshape(128, 2 * ET)
    wf[0, WF_FCB:WF_FCB + 2] = fcb / NC_

    xp = np.ascontiguousarray(
        x.reshape(KB, 128, F_IN).transpose(1, 0, 2)).astype(fp8)

    reps = {"wb": wb, "w8": w8, "wf": wf, "xp": xp}
    in_maps = []
    idx = np.arange(R)
    for r in range(NC_):
        cols = np.ascontiguousarray(adj[:, r * R:(r + 1) * R])
        cols[r * R + idx, idx] += 1.0   # A + I, this core's diagonal block
        adjp = np.ascontiguousarray(
            cols.reshape(KB, 128, R).transpose(1, 0, 2)).astype(fp8)
        in_maps.append({"adjp": adjp, **reps})
    return in_maps


def kernel(**inputs):
    from concourse.bass_utils import run_bass_kernel_spmd

    if "nc" not in _cache:
        _cache["nc"] = _build()
    nc = _cache["nc"]

    in_maps = _pack_inputs(inputs)
    res = run_bass_kernel_spmd(nc, in_maps, core_ids=list(range(NC_)))
    out = np.zeros(2, dtype=np.float64)
    for r in range(NC_):
        out += res.results[r]["outp"].reshape(2).astype(np.float64)
    return out.astype(np.float32)


# revision 22
# speedup vs baseline: 1.3101x; 1.0467x over previous
"""Trainium2 Bass kernel for GCN(x2) + MHA + mean + FC, sharded over 8 NeuronCores.

Sharding: 1D row partition of the 4096 nodes (512 rows/core). Each core holds
the column slice adj_hat[:, r*512:(r+1)*512] of the symmetric A+I (equal to its
row block transposed), all of x, and replicated weights. Cross-core exchanges
(on-device AllGather): degree vector, GCN1 output (dinv-prescaled fp8), K, V.

v2 changes over the 150us baseline:
- Softmax exp split across THREE engines: Act computes exp(s*ln2) straight
  from PSUM; DVE stages PSUM->SBUF copies; Pool computes 2^s via the pow ALU
  op from SBUF (Pool has no PSUM port). The 1/sqrt(hd)*log2(e) score scale is
  folded into qt at bias time, so probs = 2^score on every engine. This turns
  the attention phase from Act-bound (68us) into PE-bound (~41us).
- dinv = exp(-0.5*ln(deg)) on Act: ln/exp/relu/copy share ONE activation
  table, so the kernel performs a single table load, prewarmed at t=0.
- x shipped fp8; x1 AllGather carries dinv-prescaled fp8 (the GCN2-side
  rescale loop disappears; sender scale comes from a tiny transposed DMA of
  the core's own dinv, so the SPMD program stays rank-free); QKV projections
  run fp8 DoubleRow from x2t8.
- Adjacency DMA in 4 chunks with the degree matmul group chasing it; bulk
  weight loads are queued behind the degree roundtrip so the (single) DMA
  device is free when the latency-critical transfers arrive.
- K/V gathers staged per-rank with >=512B inner runs; no DMA issue on Pool.
Host does only slicing/packing (shard) and an 8-way sum of [2]-vector partials.
"""
import sys
sys.path.insert(0, "/opt/trn_rl_repo")
import numpy as np
import ml_dtypes

N = 4096
NC_ = 8
R = N // NC_          # 512 rows per core
KB = N // 128         # 32 node chunks
F_IN = 128
G1 = 128
G2 = 512
HEADS = 4
HD = G2 // HEADS      # 128
ET = G2 // 128        # 4 tiles of the 512-dim embedding

LN2 = float(np.log(2.0))
SCL = float(np.log2(np.e)) / float(np.sqrt(HD))  # probs = 2^(q.k) after fold

# wf (f32 misc pack) column offsets
WF_B1X4 = 0        # [128,512] b1 tiled 4x (broadcast over partitions)
WF_B2 = 512        # [128,4]
WF_BQ = 516        # [128,4]
WF_BK = 520        # [128,4]
WF_BV = 524        # [128,4]
WF_FOLD = 528      # [128,8] (Wo @ fc_w) packed per head
WF_FCB = 536       # [1,2] (bo @ fc_w + fc_b)/8 at partition 0
WF_COLS = 538

# wb (bf16 pack) column offsets
WB_W1 = 0            # [128,128]
WB_W2 = 128          # [128,512]
WB_COLS = 640

_cache = {}


def _build(sim1=False, rank=0):
    from concourse import bass, bacc, tile, mybir

    f32 = mybir.dt.float32
    bf16 = mybir.dt.bfloat16
    fp8 = mybir.dt.float8e4
    AF = mybir.ActivationFunctionType
    ALU = mybir.AluOpType
    PM = mybir.MatmulPerfMode
    AX = mybir.AxisListType
    from concourse.masks import make_identity

    nc = bacc.Bacc("TRN2", target_bir_lowering=False, debug=False,
                   num_devices=1 if sim1 else NC_)

    # ---- kernel I/O (per-core shards supplied via in_maps) ----
    adj_d = nc.dram_tensor("adjp", [128, KB, R], fp8, kind="ExternalInput")
    x_d = nc.dram_tensor("xp", [128, KB, F_IN], fp8, kind="ExternalInput")
    wb_d = nc.dram_tensor("wb", [128, WB_COLS], bf16, kind="ExternalInput")
    w8_d = nc.dram_tensor("w8", [128, ET, 3 * G2], fp8, kind="ExternalInput")
    wf_d = nc.dram_tensor("wf", [128, WF_COLS], f32, kind="ExternalInput")
    out_d = nc.dram_tensor("outp", [1, 2], f32, kind="ExternalOutput")

    dg_out = nc.dram_tensor("dg_out", [KB, 128], f32, kind="Internal",
                            addr_space="Shared")
    x1g = nc.dram_tensor("x1g", [NC_, 128, ET, G1], fp8, kind="Internal",
                         addr_space="Shared")
    kv2 = nc.dram_tensor("kv2", [NC_, 128, HEADS, 2 * R], fp8, kind="Internal",
                         addr_space="Shared")

    RG = [list(range(NC_))]

    with tile.TileContext(nc) as tc:
        with tc.tile_pool(name="wts", bufs=1) as wts, \
             tc.tile_pool(name="act", bufs=1) as actp, \
             tc.tile_pool(name="stg", bufs=2) as stg, \
             tc.tile_pool(name="sce", bufs=3) as sce, \
             tc.tile_pool(name="ptp", bufs=2) as ptp, \
             tc.tile_pool(name="psc", bufs=4, space="PSUM") as psc, \
             tc.tile_pool(name="dram", bufs=1, space="DRAM") as drp:

            # ================= constants + act-table prewarm =================
            ones2 = wts.tile([128, 2, 128], fp8)
            nc.vector.memset(ones2[:, :, :], 1.0)
            ident = wts.tile([32, 32], f32)
            make_identity(nc, ident[:, :])
            two1k = wts.tile([128, 1024], f32)
            nc.gpsimd.memset(two1k[:, :], float(2.0 ** SCL))
            # prewarm the (ln,exp,relu,copy) table while the adjacency loads
            warm = wts.tile([1, 1], f32)
            nc.vector.memset(warm[:, :], 1.0)
            wsc = stg.tile([1, 1], f32, tag="warm")
            nc.scalar.activation(wsc[:, :], warm[:, :], AF.Sqrt)

            # ---- input loads. adj first (degree chases it); x on the scalar
            # queue; bulk weights queued on sync BEHIND the degree roundtrip
            # so the DMA device is free for the latency-critical hops.
            adj8 = wts.tile([128, KB, R], fp8)
            for i in range(4):
                nc.sync.dma_start(adj8[:, 8 * i:8 * i + 8, :],
                                  adj_d[:, 8 * i:8 * i + 8, :])
            x8 = wts.tile([128, KB, F_IN], fp8)
            for i in range(2):
                nc.sync.dma_start(x8[:, 16 * i:16 * i + 16, :],
                                  x_d[:, 16 * i:16 * i + 16, :])
            wft = wts.tile([128, WF_COLS], f32)
            nc.sync.dma_start(wft[:], wf_d[:, :])

            # ================= degree (PE chases the adjacency DMA) ========
            ps_deg = psc.tile([128, 1024], f32, tag="c")
            for c in range(KB // 2):
                nc.tensor.matmul(ps_deg[:, 0:R], ones2[:, :, :],
                                 adj8[:, 2 * c:2 * c + 2, :],
                                 start=(c == 0), stop=(c == KB // 2 - 1),
                                 perf_mode=PM.DoubleRow)
            # dinv = 1/sqrt(deg); deg >= 1 always (A+I)
            dsq = stg.tile([128, R], f32, tag="dsq")
            nc.scalar.activation(dsq[:], ps_deg[:, 0:R], AF.Sqrt)
            dbc = wts.tile([128, R], f32)
            nc.vector.reciprocal(dbc[:], dsq[:])

            # ================= degree AllGather -> dcol =================
            if sim1:
                nc.sync.dma_start(dg_out[4 * rank:4 * rank + 4, :].flatten(),
                                  dbc[0:1, :].flatten())
                down_src = dg_out[4 * rank:4 * rank + 4, :].flatten()
            else:
                dg_in = drp.tile([1, R], f32, tag="dgin")
                nc.sync.dma_start(dg_in[:], dbc[0:1, :])
                nc.gpsimd.collective_compute(
                    "AllGather", ALU.bypass, replica_groups=RG,
                    ins=[dg_in.opt()], outs=[dg_out.ap()])
                down_src = dg_in[0, :]
            dg_sb = stg.tile([KB, 128], f32, tag="dgsb")
            nc.sync.dma_start(dg_sb[:, :], dg_out[:, :])
            # own dinv transposed to [node%128, mt] for the x1 sender scale
            down = wts.tile([128, ET], f32)
            nc.sync.dma_start(down[:, :],
                              down_src.rearrange("(mt p) -> p mt", p=128))
            # bulk weights now (device free until x1 AG)
            wbt = wts.tile([128, WB_COLS], bf16)
            nc.sync.dma_start(wbt[:], wb_d[:, :])
            w8t = wts.tile([128, ET, 3 * G2], fp8)
            nc.sync.dma_start(w8t[:, :, :], w8_d[:, :, :])

            ps_t = psc.tile([128, 1024], f32, tag="c")
            nc.tensor.transpose(ps_t[:, 0:KB], dg_sb[:, :], ident[:, :])
            dcol = wts.tile([128, KB], f32)
            nc.vector.tensor_copy(dcol[:], ps_t[:, 0:KB])

            # ================= GCN1 =================
            # xs8[:,kb,:] = x8[:,kb,:] * dinv[node]; 3-way engine split
            xs8 = actp.tile([128, KB, F_IN], fp8)
            for kb in range(KB):
                e = kb % 3
                if e == 0:
                    nc.vector.tensor_scalar_mul(xs8[:, kb, :], x8[:, kb, :],
                                                dcol[:, kb:kb + 1])
                elif e == 1:
                    nc.gpsimd.tensor_scalar_mul(xs8[:, kb, :], x8[:, kb, :],
                                                dcol[:, kb:kb + 1])
                else:
                    nc.scalar.activation(xs8[:, kb, :], x8[:, kb, :],
                                         AF.Copy, scale=dcol[:, kb:kb + 1])
            ps_s1 = psc.tile([128, 1024], f32, tag="c")
            for c in range(KB // 2):
                nc.tensor.matmul(ps_s1[:, 0:R], xs8[:, 2 * c:2 * c + 2, :],
                                 adj8[:, 2 * c:2 * c + 2, :],
                                 start=(c == 0), stop=(c == KB // 2 - 1),
                                 perf_mode=PM.DoubleRow)
            s1t = actp.tile([128, R], bf16)
            nc.vector.tensor_mul(s1t[:], ps_s1[:, 0:R], dbc[:])

            # W1 + bias, then relu with own-dinv prescale -> fp8 for the AG
            ps_x1 = psc.tile([128, 1024], f32, tag="c")
            for mt in range(4):
                nc.tensor.matmul(ps_x1[:, mt * G1:(mt + 1) * G1],
                                 s1t[:, mt * 128:(mt + 1) * 128],
                                 wbt[:, WB_W1:WB_W1 + G1],
                                 start=True, stop=True, skip_group_check=True)
            x1b = stg.tile([128, 4 * G1], f32, tag="x1b")
            x1s = actp.tile([128, ET, G1], fp8)
            for mt in range(4):
                nc.vector.tensor_add(x1b[:, mt * G1:(mt + 1) * G1],
                                     ps_x1[:, mt * G1:(mt + 1) * G1],
                                     wft[:, WF_B1X4 + mt * G1:WF_B1X4 + (mt + 1) * G1])
                # relu(x+b)*d == relu((x+b)*d) since d > 0
                nc.scalar.activation(x1s[:, mt, :],
                                     x1b[:, mt * G1:(mt + 1) * G1], AF.Relu,
                                     scale=down[:, mt:mt + 1])
            nc.scalar.activation(wsc[:, :], x1s[0:1, ET - 1, 0:1], AF.Exp)
            if sim1:
                nc.sync.dma_start(x1g[rank, :, :, :], x1s[:, :, :])
            else:
                x1_in = drp.tile([128, ET, G1], fp8, tag="x1in")
                nc.sync.dma_start(x1_in[:, :, :], x1s[:, :, :])
                nc.gpsimd.collective_compute(
                    "AllGather", ALU.bypass, replica_groups=RG,
                    ins=[x1_in.opt()], outs=[x1g.ap()])

            # ================= GCN2 =================
            x1a = actp.tile([128, NC_, ET, G1], fp8)
            if sim1:
                nc.sync.dma_start(
                    x1a[:, :, :, :],
                    x1g[rank, :, :, :].unsqueeze(1).broadcast_to(
                        [128, NC_, ET, G1]))
            else:
                nc.sync.dma_start(x1a[:, :, :, :],
                                  x1g[:, :, :, :].transpose([1, 0, 2, 3]))
            ps_s2 = psc.tile([128, 1024], f32, tag="c")
            for c in range(KB // 2):
                r, mm = (2 * c) // 4, (2 * c) % 4
                nc.tensor.matmul(ps_s2[:, 0:R], x1a[:, r, mm:mm + 2, :],
                                 adj8[:, 2 * c:2 * c + 2, :],
                                 start=(c == 0), stop=(c == KB // 2 - 1),
                                 perf_mode=PM.DoubleRow)
            s2t = actp.tile([128, R], bf16)
            nc.vector.tensor_mul(s2t[:], ps_s2[:, 0:R], dbc[:])

            # W2: x2t8[e-tile, node] fp8 (+bias), feeds QKV via DoubleRow
            x2t8 = actp.tile([128, ET, R], fp8)
            for eo in range(2):
                ps_x2 = psc.tile([128, 1024], f32, tag="c")
                for ei in range(2):
                    et = 2 * eo + ei
                    nc.tensor.matmul(ps_x2[:, ei * R:(ei + 1) * R],
                                     wbt[:, WB_W2 + et * 128:WB_W2 + (et + 1) * 128],
                                     s2t[:], start=True, stop=True,
                                     skip_group_check=True)
                for ei in range(2):
                    et = 2 * eo + ei
                    if ei == 0:
                        nc.vector.tensor_scalar_add(
                            x2t8[:, et, :], ps_x2[:, ei * R:(ei + 1) * R],
                            wft[:, WF_B2 + et:WF_B2 + et + 1])
                    else:
                        nc.scalar.activation(
                            x2t8[:, et, :], ps_x2[:, ei * R:(ei + 1) * R],
                            AF.Identity,
                            bias=wft[:, WF_B2 + et:WF_B2 + et + 1])

            # ================= QKV (fp8 DoubleRow) =================
            qt = actp.tile([128, HEADS, R], fp8)
            kvl = actp.tile([128, HEADS, 2 * R], fp8)
            for hp in range(2):
                # K heads pair-wise first so the K AllGather starts early
                ps_qk = psc.tile([128, 1024], f32, tag="c")
                for hh in range(2):
                    h = 2 * hp + hh
                    for cp in range(2):
                        nc.tensor.matmul(
                            ps_qk[:, hh * R:(hh + 1) * R],
                            w8t[:, 2 * cp:2 * cp + 2, G2 + h * 128:G2 + (h + 1) * 128],
                            x2t8[:, 2 * cp:2 * cp + 2, :],
                            start=(cp == 0), stop=(cp == 1),
                            perf_mode=PM.DoubleRow, skip_group_check=True)
                for hh in range(2):
                    h = 2 * hp + hh
                    if hh == 0:
                        nc.vector.tensor_scalar_add(
                            kvl[:, h, 0:R], ps_qk[:, hh * R:(hh + 1) * R],
                            wft[:, WF_BK + h:WF_BK + h + 1])
                    else:
                        nc.scalar.activation(
                            kvl[:, h, 0:R], ps_qk[:, hh * R:(hh + 1) * R],
                            AF.Identity,
                            bias=wft[:, WF_BK + h:WF_BK + h + 1])

            for hp in range(2):
                ps_qk = psc.tile([128, 1024], f32, tag="c")
                for hh in range(2):
                    h = 2 * hp + hh
                    for cp in range(2):
                        nc.tensor.matmul(
                            ps_qk[:, hh * R:(hh + 1) * R],
                            w8t[:, 2 * cp:2 * cp + 2, h * 128:(h + 1) * 128],
                            x2t8[:, 2 * cp:2 * cp + 2, :],
                            start=(cp == 0), stop=(cp == 1),
                            perf_mode=PM.DoubleRow, skip_group_check=True)
                for hh in range(2):
                    h = 2 * hp + hh
                    # plain q + bq; the 1/sqrt(hd) softmax scale is folded
                    # into the exp bases (act scale / pool pow base)
                    if hh == 0:
                        nc.vector.tensor_scalar_add(
                            qt[:, h, :], ps_qk[:, hh * R:(hh + 1) * R],
                            wft[:, WF_BQ + h:WF_BQ + h + 1])
                    else:
                        nc.scalar.activation(
                            qt[:, h, :], ps_qk[:, hh * R:(hh + 1) * R],
                            AF.Identity,
                            bias=wft[:, WF_BQ + h:WF_BQ + h + 1])

            for h in range(HEADS):
                ps_v = psc.tile([128, 1024], f32, tag="c")
                for mt in range(4):
                    for cp in range(2):
                        nc.tensor.matmul(
                            ps_v[:, mt * HD:(mt + 1) * HD],
                            x2t8[:, 2 * cp:2 * cp + 2, mt * 128:(mt + 1) * 128],
                            w8t[:, 2 * cp:2 * cp + 2,
                                2 * G2 + h * HD:2 * G2 + (h + 1) * HD],
                            start=(cp == 0), stop=(cp == 1),
                            perf_mode=PM.DoubleRow, skip_group_check=True)
                if h % 2 == 0:
                    nc.vector.tensor_copy(kvl[:, h, R:2 * R], ps_v[:, 0:4 * HD])
                else:
                    nc.scalar.activation(kvl[:, h, R:2 * R], ps_v[:, 0:4 * HD],
                                         AF.Copy)
            if sim1:
                nc.sync.dma_start(kv2[rank, :, :, :], kvl[:, :, :])
            else:
                kvi = drp.tile([128, HEADS, 2 * R], fp8, tag="kvi")
                nc.sync.dma_start(kvi[:, :, :], kvl[:, :, :])
                nc.gpsimd.collective_compute(
                    "AllGather", ALU.bypass, replica_groups=RG,
                    ins=[kvi.opt()], outs=[kv2.ap()])

            # stage gathered K/V into SBUF, one DMA per rank (inner runs big).
            # K pieces first: scores(h0) chases them.
            ktg = actp.tile([128, NC_, HEADS, R], fp8)
            vgl = actp.tile([128, NC_, HEADS, 4, HD], fp8)
            for r in range(NC_):
                src_r = rank if sim1 else r
                eng = [nc.sync, nc.scalar][r % 2]
                eng.dma_start(ktg[:, r, :, :], kv2[src_r, :, :, 0:R])
            for r in range(NC_):
                src_r = rank if sim1 else r
                eng = [nc.sync, nc.scalar][r % 2]
                eng.dma_start(
                    vgl[:, r, :, :, :],
                    kv2[src_r, :, :, R:2 * R].rearrange(
                        "p h (a b) -> p h a b", a=4))

            # ================= attention =================
            zb = wts.tile([128, HEADS], f32)
            nc.vector.tensor_scalar_mul(zb[:], wft[:, WF_BV:WF_BV + HEADS],
                                        float(R))
            zf = actp.tile([128, HEADS], f32)
            junk = wts.tile([128, R], f32)
            pts = {}
            cds = {}

            def pass2(hh):
                # denominator then context: contiguous DoubleRow groups
                ps_cd = psc.tile([128, 1024], f32, tag="c")
                cds[hh] = ps_cd
                pth = pts[hh]
                for pc in range(KB // 2):
                    nc.tensor.matmul(ps_cd[:, R:2 * R], ones2[:, :, :],
                                     pth[:, 2 * pc:2 * pc + 2, :],
                                     start=(pc == 0), stop=(pc == KB // 2 - 1),
                                     perf_mode=PM.DoubleRow,
                                     skip_group_check=True)
                for pc in range(KB // 2):
                    r, mm = (2 * pc) // 4, (2 * pc) % 4
                    nc.tensor.matmul(ps_cd[:, 0:R],
                                     vgl[:, r, hh, mm:mm + 2, :],
                                     pth[:, 2 * pc:2 * pc + 2, :],
                                     start=(pc == 0), stop=(pc == KB // 2 - 1),
                                     perf_mode=PM.DoubleRow,
                                     skip_group_check=True)

            zb16 = actp.tile([128, HEADS], f32)

            def tail(hh):
                ps_cd = cds[hh]
                rbc = stg.tile([128, R], f32, tag="rbc")
                nc.vector.reciprocal(rbc[:], ps_cd[:, R:2 * R])
                zr = stg.tile([128, 1], f32, tag="zr")
                nc.vector.scalar_tensor_tensor(
                    junk[:], ps_cd[:, 0:R], 0.0, rbc[:],
                    ALU.bypass, ALU.mult, accum_out=zr[:])
                nc.gpsimd.tensor_add(zf[:, hh:hh + 1], zr[:], zb[:, hh:hh + 1])
                # fold this head's context into the out_proj accumulation
                nc.gpsimd.tensor_scalar_mul(zb16[:, hh:hh + 1],
                                            zf[:, hh:hh + 1], 1.0 / float(N))

            # exp chunk routing: per head 16 chunks of [128,1024];
            # 9 -> Act (exp from PSUM), 7 -> DVE copy + Pool 2^x
            POOL_SETS = [{2, 5, 7, 10, 12, 15}, {1, 3, 5, 8, 10, 12, 14}]

            def score_pair(h, pc):
                pt = pts[h]
                mc0, mc1 = 2 * pc, 2 * pc + 1
                ps_sc = psc.tile([128, 1024], f32, tag="c")
                nc.tensor.matmul(
                    ps_sc[:, 0:R],
                    ktg[:, mc0 // 4, h,
                        (mc0 % 4) * 128:(mc0 % 4) * 128 + 128],
                    qt[:, h, :], start=True, stop=True,
                    skip_group_check=True)
                nc.tensor.matmul(
                    ps_sc[:, R:2 * R],
                    ktg[:, mc1 // 4, h,
                        (mc1 % 4) * 128:(mc1 % 4) * 128 + 128],
                    qt[:, h, :], start=True, stop=True,
                    skip_group_check=True)
                if pc in POOL_SETS[h % 2]:
                    st = sce.tile([128, 1024], f32, tag="st")
                    nc.vector.tensor_copy(st[:, :], ps_sc[:, 0:1024])
                    nc.gpsimd.tensor_tensor(pt[:, 2 * pc:2 * pc + 2, :],
                                            two1k[:, :], st[:, :], ALU.pow)
                else:
                    nc.scalar.activation(pt[:, 2 * pc:2 * pc + 2, :],
                                         ps_sc[:, 0:1024], AF.Exp,
                                         scale=LN2 * SCL)

            # interleave prev head's den/ctx DR groups between score chunks
            # so the exp engines never starve at head boundaries
            for h in range(HEADS):
                pt_h = ptp.tile([128, KB, R], fp8, tag="pt")
                pts[h] = pt_h
                for pc in range(8):
                    score_pair(h, pc)
                if h >= 1:
                    pass2_den(h - 1)
                for pc in range(8, 12):
                    score_pair(h, pc)
                if h >= 1:
                    pass2_ctx(h - 1)
                for pc in range(12, KB // 2):
                    score_pair(h, pc)
                if h >= 1:
                    tail(h - 1)
            pass2_den(HEADS - 1)
            pass2_ctx(HEADS - 1)
            tail(HEADS - 1)

            # ========== folded out_proj@fc: out = (sum zf/N) @ Wfold ==========
            ps_fc = psc.tile([128, 1024], f32, tag="c")
            for h in range(HEADS):
                nc.tensor.matmul(ps_fc[0:1, 0:2], zb16[:, h:h + 1],
                                 wft[:, WF_FOLD + 2 * h:WF_FOLD + 2 * h + 2],
                                 start=(h == 0), stop=(h == HEADS - 1),
                                 skip_group_check=True)
            ores = stg.tile([1, 2], f32, tag="ores")
            nc.vector.tensor_add(ores[:], ps_fc[0:1, 0:2],
                                 wft[0:1, WF_FCB:WF_FCB + 2])
            nc.sync.dma_start(out_d[:, :], ores[:])

    nc.compile()
    return nc


def _pack_inputs(inputs):
    """Pack full inputs into per-core shards + replicated weight blocks."""
    fp8 = ml_dtypes.float8_e4m3
    bf16 = ml_dtypes.bfloat16

    adj = np.ascontiguousarray(inputs["adj_matrix"], dtype=np.float32)
    x = np.ascontiguousarray(inputs["node_features"], dtype=np.float32)
    W1 = np.asarray(inputs["W1"], np.float32)
    b1 = np.asarray(inputs["b1"], np.float32)
    W2 = np.asarray(inputs["W2"], np.float32)
    b2 = np.asarray(inputs["b2"], np.float32)
    ipw = np.asarray(inputs["in_proj_w"], np.float32)
    ipb = np.asarray(inputs["in_proj_b"], np.float32)
    wo = np.asarray(inputs["out_proj_w"], np.float32)
    bo = np.asarray(inputs["out_proj_b"], np.float32)
    fcw = np.asarray(inputs["fc_w"], np.float32)
    fcb = np.asarray(inputs["fc_b"], np.float32)

    wb = np.zeros((128, WB_COLS), np.float32)
    wb[:, WB_W1:WB_W1 + G1] = W1
    wb[:, WB_W2:WB_W2 + G2] = W2
    wb = wb.astype(bf16)

    # fp8 QKV weights: [128, c-tile, 3*G2] (q | k | v)
    w8 = np.zeros((128, ET, 3 * G2), np.float32)
    for c in range(ET):
        w8[:, c, :] = ipw[c * 128:(c + 1) * 128, :]
    w8 = w8.astype(fp8)

    wf = np.zeros((128, WF_COLS), np.float32)
    wf[:, WF_B1X4:WF_B1X4 + 4 * G1] = np.tile(b1, 4)[None, :]
    wf[:, WF_B2:WF_B2 + ET] = b2.reshape(ET, 128).T
    wf[:, WF_BQ:WF_BQ + HEADS] = ipb[0:G2].reshape(HEADS, HD).T
    wf[:, WF_BK:WF_BK + HEADS] = ipb[G2:2 * G2].reshape(HEADS, HD).T
    wf[:, WF_BV:WF_BV + HEADS] = ipb[2 * G2:3 * G2].reshape(HEADS, HD).T
    wfold = (wo.astype(np.float64) @ fcw.astype(np.float64)).astype(np.float32)
    wf[:, WF_FOLD:WF_FOLD + 2 * HEADS] = wfold.reshape(HEADS, HD, 2) \
        .transpose(1, 0, 2).reshape(HD, 2 * HEADS)
    wf[0, WF_FCB:WF_FCB + 2] = (bo.astype(np.float64) @ fcw.astype(np.float64)
                                + fcb).astype(np.float32) / NC_

    xp = np.ascontiguousarray(
        x.reshape(KB, 128, F_IN).transpose(1, 0, 2)).astype(fp8)

    reps = {"wb": wb, "w8": w8, "wf": wf, "xp": xp}
    in_maps = []
    idx = np.arange(R)
    for r in range(NC_):
        cols = np.ascontiguousarray(adj[:, r * R:(r + 1) * R])
        cols[r * R + idx, idx] += 1.0   # A + I, this core's diagonal block
        adjp = np.ascontiguousarray(
            cols.reshape(KB, 128, R).transpose(1, 0, 2)).astype(fp8)
        in_maps.append({"adjp": adjp, **reps})
    return in_maps


def kernel(**inputs):
    from concourse.bass_utils import run_bass_kernel_spmd

    if "nc" not in _cache:
        _cache["nc"] = _build()
    nc = _cache["nc"]

    in_maps = _pack_inputs(inputs)
    res = run_bass_kernel_spmd(nc, in_maps, core_ids=list(range(NC_)))
    out = np.zeros(2, dtype=np.float64)
    for r in range(NC_):
        out += res.results[r]["outp"].reshape(2).astype(np.float64)
    return out.astype(np.float32)


# revision 28
# speedup vs baseline: 1.3558x; 1.0349x over previous
"""Trainium2 Bass kernel for GCN(x2) + MHA + mean + FC, sharded over 8 NeuronCores.

Sharding: 1D row partition of the 4096 nodes (512 rows/core). Each core holds
the column slice adj_hat[:, r*512:(r+1)*512] of the symmetric A+I (equal to its
row block transposed), all of x, and replicated weights. Cross-core exchanges
(on-device AllGather): degree vector, GCN1 output (dinv-prescaled fp8), K, V.

v2 changes over the 150us baseline:
- Softmax exp split across THREE engines: Act computes exp(s*ln2) straight
  from PSUM; DVE stages PSUM->SBUF copies; Pool computes 2^s via the pow ALU
  op from SBUF (Pool has no PSUM port). The 1/sqrt(hd)*log2(e) score scale is
  folded into qt at bias time, so probs = 2^score on every engine. This turns
  the attention phase from Act-bound (68us) into PE-bound (~41us).
- dinv = exp(-0.5*ln(deg)) on Act: ln/exp/relu/copy share ONE activation
  table, so the kernel performs a single table load, prewarmed at t=0.
- x shipped fp8; x1 AllGather carries dinv-prescaled fp8 (the GCN2-side
  rescale loop disappears; sender scale comes from a tiny transposed DMA of
  the core's own dinv, so the SPMD program stays rank-free); QKV projections
  run fp8 DoubleRow from x2t8.
- Adjacency DMA in 4 chunks with the degree matmul group chasing it; bulk
  weight loads are queued behind the degree roundtrip so the (single) DMA
  device is free when the latency-critical transfers arrive.
- K/V gathers staged per-rank with >=512B inner runs; no DMA issue on Pool.
Host does only slicing/packing (shard) and an 8-way sum of [2]-vector partials.
"""
import sys
sys.path.insert(0, "/opt/trn_rl_repo")
import numpy as np
import ml_dtypes

N = 4096
NC_ = 8
R = N // NC_          # 512 rows per core
KB = N // 128         # 32 node chunks
F_IN = 128
G1 = 128
G2 = 512
HEADS = 4
HD = G2 // HEADS      # 128
ET = G2 // 128        # 4 tiles of the 512-dim embedding

LN2 = float(np.log(2.0))
SCL = float(np.log2(np.e)) / float(np.sqrt(HD))  # probs = 2^(q.k) after fold

# wf (f32 misc pack) column offsets
WF_B1X4 = 0        # [128,512] b1 tiled 4x (broadcast over partitions)
WF_B2 = 512        # [128,4]
WF_BQ = 516        # [128,4]
WF_BK = 520        # [128,4]
WF_BV = 524        # [128,4]
WF_FOLD = 528      # [128,8] (Wo @ fc_w) packed per head
WF_FCB = 536       # [1,2] (bo @ fc_w + fc_b)/8 at partition 0
WF_COLS = 538

# wb (bf16 pack) column offsets
WB_W1 = 0            # [128,128]
WB_W2 = 128          # [128,512]
WB_COLS = 640

_cache = {}


def _build(sim1=False, rank=0):
    from concourse import bass, bacc, tile, mybir

    f32 = mybir.dt.float32
    bf16 = mybir.dt.bfloat16
    fp8 = mybir.dt.float8e4
    AF = mybir.ActivationFunctionType
    ALU = mybir.AluOpType
    PM = mybir.MatmulPerfMode
    AX = mybir.AxisListType
    from concourse.masks import make_identity

    nc = bacc.Bacc("TRN2", target_bir_lowering=False, debug=False,
                   num_devices=1 if sim1 else NC_)

    # ---- kernel I/O (per-core shards supplied via in_maps) ----
    adj_d = nc.dram_tensor("adjp", [128, KB, R], fp8, kind="ExternalInput")
    x_d = nc.dram_tensor("xp", [128, KB, F_IN], fp8, kind="ExternalInput")
    wb_d = nc.dram_tensor("wb", [128, WB_COLS], bf16, kind="ExternalInput")
    w8_d = nc.dram_tensor("w8", [128, ET, 3 * G2], fp8, kind="ExternalInput")
    wf_d = nc.dram_tensor("wf", [128, WF_COLS], f32, kind="ExternalInput")
    out_d = nc.dram_tensor("outp", [1, 2], f32, kind="ExternalOutput")

    dg_out = nc.dram_tensor("dg_out", [KB, 128], f32, kind="Internal",
                            addr_space="Shared")
    x1g = nc.dram_tensor("x1g", [NC_, 128, ET, G1], fp8, kind="Internal",
                         addr_space="Shared")
    kv2 = nc.dram_tensor("kv2", [NC_, 128, HEADS, 2 * R], fp8, kind="Internal",
                         addr_space="Shared")

    RG = [list(range(NC_))]

    with tile.TileContext(nc) as tc:
        with tc.tile_pool(name="wts", bufs=1) as wts, \
             tc.tile_pool(name="act", bufs=1) as actp, \
             tc.tile_pool(name="stg", bufs=2) as stg, \
             tc.tile_pool(name="sce", bufs=3) as sce, \
             tc.tile_pool(name="ptp", bufs=2) as ptp, \
             tc.tile_pool(name="psc", bufs=3, space="PSUM") as psc, \
             tc.tile_pool(name="pscd", bufs=1, space="PSUM") as pscd, \
             tc.tile_pool(name="dram", bufs=1, space="DRAM") as drp:

            # ================= constants + act-table prewarm =================
            ones2 = wts.tile([128, 2, 128], fp8)
            nc.vector.memset(ones2[:, :, :], 1.0)
            ident = wts.tile([32, 32], f32)
            make_identity(nc, ident[:, :])
            two1k = wts.tile([128, 1024], f32)
            nc.gpsimd.memset(two1k[:, :], float(2.0 ** SCL))
            # prewarm the (ln,exp,relu,copy) table while the adjacency loads
            warm = wts.tile([1, 1], f32)
            nc.vector.memset(warm[:, :], 1.0)
            wsc = stg.tile([1, 1], f32, tag="warm")
            nc.scalar.activation(wsc[:, :], warm[:, :], AF.Sqrt)

            # ---- input loads. adj first (degree chases it); x on the scalar
            # queue; bulk weights queued on sync BEHIND the degree roundtrip
            # so the DMA device is free for the latency-critical hops.
            adj8 = wts.tile([128, KB, R], fp8)
            for i in range(4):
                nc.sync.dma_start(adj8[:, 8 * i:8 * i + 8, :],
                                  adj_d[:, 8 * i:8 * i + 8, :])
            x8 = wts.tile([128, KB, F_IN], fp8)
            for i in range(2):
                nc.sync.dma_start(x8[:, 16 * i:16 * i + 16, :],
                                  x_d[:, 16 * i:16 * i + 16, :])
            wft = wts.tile([128, WF_COLS], f32)
            nc.sync.dma_start(wft[:], wf_d[:, :])

            # ================= degree (PE chases the adjacency DMA) ========
            ps_deg = psc.tile([128, 1024], f32, tag="c")
            for c in range(KB // 2):
                nc.tensor.matmul(ps_deg[:, 0:R], ones2[:, :, :],
                                 adj8[:, 2 * c:2 * c + 2, :],
                                 start=(c == 0), stop=(c == KB // 2 - 1),
                                 perf_mode=PM.DoubleRow)
            # dinv = 1/sqrt(deg); deg >= 1 always (A+I)
            dsq = stg.tile([128, R], f32, tag="dsq")
            nc.scalar.activation(dsq[:], ps_deg[:, 0:R], AF.Sqrt)
            dbc = wts.tile([128, R], f32)
            nc.vector.reciprocal(dbc[:], dsq[:])

            # ================= degree AllGather -> dcol =================
            if sim1:
                nc.sync.dma_start(dg_out[4 * rank:4 * rank + 4, :].flatten(),
                                  dbc[0:1, :].flatten())
                down_src = dg_out[4 * rank:4 * rank + 4, :].flatten()
            else:
                dg_in = drp.tile([1, R], f32, tag="dgin")
                nc.sync.dma_start(dg_in[:], dbc[0:1, :])
                nc.gpsimd.collective_compute(
                    "AllGather", ALU.bypass, replica_groups=RG,
                    ins=[dg_in.opt()], outs=[dg_out.ap()])
                down_src = dg_in[0, :]
            dg_sb = stg.tile([KB, 128], f32, tag="dgsb")
            nc.sync.dma_start(dg_sb[:, :], dg_out[:, :])
            # own dinv transposed to [node%128, mt] for the x1 sender scale
            down = wts.tile([128, ET], f32)
            nc.sync.dma_start(down[:, :],
                              down_src.rearrange("(mt p) -> p mt", p=128))
            # bulk weights now (device free until x1 AG)
            wbt = wts.tile([128, WB_COLS], bf16)
            nc.sync.dma_start(wbt[:], wb_d[:, :])
            w8t = wts.tile([128, ET, 3 * G2], fp8)
            nc.sync.dma_start(w8t[:, :, :], w8_d[:, :, :])

            ps_t = psc.tile([128, 1024], f32, tag="c")
            nc.tensor.transpose(ps_t[:, 0:KB], dg_sb[:, :], ident[:, :])
            dcol = wts.tile([128, KB], f32)
            nc.vector.tensor_copy(dcol[:], ps_t[:, 0:KB])

            # ================= GCN1 =================
            # xs8[:,kb,:] = x8[:,kb,:] * dinv[node]; 3-way engine split
            xs8 = actp.tile([128, KB, F_IN], fp8)
            for kb in range(KB):
                e = kb % 3
                if e == 0:
                    nc.vector.tensor_scalar_mul(xs8[:, kb, :], x8[:, kb, :],
                                                dcol[:, kb:kb + 1])
                elif e == 1:
                    nc.gpsimd.tensor_scalar_mul(xs8[:, kb, :], x8[:, kb, :],
                                                dcol[:, kb:kb + 1])
                else:
                    nc.scalar.activation(xs8[:, kb, :], x8[:, kb, :],
                                         AF.Copy, scale=dcol[:, kb:kb + 1])
            ps_w1 = pscd.tile([128, 1024], f32, tag="d")
            for i in range(40):
                nc.tensor.matmul(ps_w1[0:1, 0:1], dcol[:, 0:1], dcol[:, 0:1],
                                 start=True, stop=True, skip_group_check=True)
            ps_s1 = psc.tile([128, 1024], f32, tag="c")
            for c in range(KB // 2):
                nc.tensor.matmul(ps_s1[:, 0:R], xs8[:, 2 * c:2 * c + 2, :],
                                 adj8[:, 2 * c:2 * c + 2, :],
                                 start=(c == 0), stop=(c == KB // 2 - 1),
                                 perf_mode=PM.DoubleRow)
            s1t = actp.tile([128, R], bf16)
            nc.vector.tensor_mul(s1t[:], ps_s1[:, 0:R], dbc[:])

            # W1 + bias, then relu with own-dinv prescale -> fp8 for the AG
            ps_x1 = psc.tile([128, 1024], f32, tag="c")
            for mt in range(4):
                nc.tensor.matmul(ps_x1[:, mt * G1:(mt + 1) * G1],
                                 s1t[:, mt * 128:(mt + 1) * 128],
                                 wbt[:, WB_W1:WB_W1 + G1],
                                 start=True, stop=True, skip_group_check=True)
            x1b = stg.tile([128, 4 * G1], f32, tag="x1b")
            x1s = actp.tile([128, ET, G1], fp8)
            for mt in range(4):
                nc.vector.tensor_add(x1b[:, mt * G1:(mt + 1) * G1],
                                     ps_x1[:, mt * G1:(mt + 1) * G1],
                                     wft[:, WF_B1X4 + mt * G1:WF_B1X4 + (mt + 1) * G1])
                # relu(x+b)*d == relu((x+b)*d) since d > 0
                nc.scalar.activation(x1s[:, mt, :],
                                     x1b[:, mt * G1:(mt + 1) * G1], AF.Relu,
                                     scale=down[:, mt:mt + 1])
            nc.scalar.activation(wsc[:, :], x1s[0:1, ET - 1, 0:1], AF.Exp)
            if sim1:
                nc.sync.dma_start(x1g[rank, :, :, :], x1s[:, :, :])
            else:
                x1_in = drp.tile([128, ET, G1], fp8, tag="x1in")
                nc.sync.dma_start(x1_in[:, :, :], x1s[:, :, :])
                nc.gpsimd.collective_compute(
                    "AllGather", ALU.bypass, replica_groups=RG,
                    ins=[x1_in.opt()], outs=[x1g.ap()])

            # ================= GCN2 =================
            x1a = actp.tile([128, NC_, ET, G1], fp8)
            if sim1:
                nc.sync.dma_start(
                    x1a[:, :, :, :],
                    x1g[rank, :, :, :].unsqueeze(1).broadcast_to(
                        [128, NC_, ET, G1]))
            else:
                nc.sync.dma_start(x1a[:, :, :, :],
                                  x1g[:, :, :, :].transpose([1, 0, 2, 3]))
            ps_w = pscd.tile([128, 1024], f32, tag="d")
            for i in range(40):
                nc.tensor.matmul(ps_w[0:1, 0:2], x1a[:, 0, 0:1, 0:1],
                                 x1a[:, 0, 0:1, 0:2], start=True, stop=True,
                                 skip_group_check=True)
            ps_s2 = psc.tile([128, 1024], f32, tag="c")
            for c in range(KB // 2):
                r, mm = (2 * c) // 4, (2 * c) % 4
                nc.tensor.matmul(ps_s2[:, 0:R], x1a[:, r, mm:mm + 2, :],
                                 adj8[:, 2 * c:2 * c + 2, :],
                                 start=(c == 0), stop=(c == KB // 2 - 1),
                                 perf_mode=PM.DoubleRow)
            s2t = actp.tile([128, R], bf16)
            nc.vector.tensor_mul(s2t[:], ps_s2[:, 0:R], dbc[:])

            # W2: x2t8[e-tile, node] fp8 (+bias), feeds QKV via DoubleRow
            x2t8 = actp.tile([128, ET, R], fp8)
            for eo in range(2):
                ps_x2 = psc.tile([128, 1024], f32, tag="c")
                for ei in range(2):
                    et = 2 * eo + ei
                    nc.tensor.matmul(ps_x2[:, ei * R:(ei + 1) * R],
                                     wbt[:, WB_W2 + et * 128:WB_W2 + (et + 1) * 128],
                                     s2t[:], start=True, stop=True,
                                     skip_group_check=True)
                for ei in range(2):
                    et = 2 * eo + ei
                    if ei == 0:
                        nc.vector.tensor_scalar_add(
                            x2t8[:, et, :], ps_x2[:, ei * R:(ei + 1) * R],
                            wft[:, WF_B2 + et:WF_B2 + et + 1])
                    else:
                        nc.scalar.activation(
                            x2t8[:, et, :], ps_x2[:, ei * R:(ei + 1) * R],
                            AF.Identity,
                            bias=wft[:, WF_B2 + et:WF_B2 + et + 1])

            # ================= QKV (fp8 DoubleRow) =================
            qt = actp.tile([128, HEADS, R], fp8)
            kvl = actp.tile([128, HEADS, 2 * R], fp8)
            for hp in range(2):
                # K heads pair-wise first so the K AllGather starts early
                ps_qk = psc.tile([128, 1024], f32, tag="c")
                for hh in range(2):
                    h = 2 * hp + hh
                    for cp in range(2):
                        nc.tensor.matmul(
                            ps_qk[:, hh * R:(hh + 1) * R],
                            w8t[:, 2 * cp:2 * cp + 2, G2 + h * 128:G2 + (h + 1) * 128],
                            x2t8[:, 2 * cp:2 * cp + 2, :],
                            start=(cp == 0), stop=(cp == 1),
                            perf_mode=PM.DoubleRow, skip_group_check=True)
                for hh in range(2):
                    h = 2 * hp + hh
                    if hh == 0:
                        nc.vector.tensor_scalar_add(
                            kvl[:, h, 0:R], ps_qk[:, hh * R:(hh + 1) * R],
                            wft[:, WF_BK + h:WF_BK + h + 1])
                    else:
                        nc.scalar.activation(
                            kvl[:, h, 0:R], ps_qk[:, hh * R:(hh + 1) * R],
                            AF.Identity,
                            bias=wft[:, WF_BK + h:WF_BK + h + 1])

            # export the K half as soon as K heads are done; V follows
            if sim1:
                nc.sync.dma_start(kv2[rank, :, :, 0:R], kvl[:, :, 0:R])
            else:
                kvi = drp.tile([128, HEADS, 2 * R], fp8, tag="kvi")
                nc.sync.dma_start(kvi[:, :, 0:R], kvl[:, :, 0:R])

            for hp in range(2):
                ps_qk = psc.tile([128, 1024], f32, tag="c")
                for hh in range(2):
                    h = 2 * hp + hh
                    for cp in range(2):
                        nc.tensor.matmul(
                            ps_qk[:, hh * R:(hh + 1) * R],
                            w8t[:, 2 * cp:2 * cp + 2, h * 128:(h + 1) * 128],
                            x2t8[:, 2 * cp:2 * cp + 2, :],
                            start=(cp == 0), stop=(cp == 1),
                            perf_mode=PM.DoubleRow, skip_group_check=True)
                for hh in range(2):
                    h = 2 * hp + hh
                    # plain q + bq; the 1/sqrt(hd) softmax scale is folded
                    # into the exp bases (act scale / pool pow base)
                    if hh == 0:
                        nc.vector.tensor_scalar_add(
                            qt[:, h, :], ps_qk[:, hh * R:(hh + 1) * R],
                            wft[:, WF_BQ + h:WF_BQ + h + 1])
                    else:
                        nc.scalar.activation(
                            qt[:, h, :], ps_qk[:, hh * R:(hh + 1) * R],
                            AF.Identity,
                            bias=wft[:, WF_BQ + h:WF_BQ + h + 1])

            for h in range(HEADS):
                ps_v = psc.tile([128, 1024], f32, tag="c")
                for mt in range(4):
                    for cp in range(2):
                        nc.tensor.matmul(
                            ps_v[:, mt * HD:(mt + 1) * HD],
                            x2t8[:, 2 * cp:2 * cp + 2, mt * 128:(mt + 1) * 128],
                            w8t[:, 2 * cp:2 * cp + 2,
                                2 * G2 + h * HD:2 * G2 + (h + 1) * HD],
                            start=(cp == 0), stop=(cp == 1),
                            perf_mode=PM.DoubleRow, skip_group_check=True)
                if h % 2 == 0:
                    nc.vector.tensor_copy(kvl[:, h, R:2 * R], ps_v[:, 0:4 * HD])
                else:
                    nc.scalar.activation(kvl[:, h, R:2 * R], ps_v[:, 0:4 * HD],
                                         AF.Copy)
            if sim1:
                nc.sync.dma_start(kv2[rank, :, :, R:2 * R], kvl[:, :, R:2 * R])
            else:
                nc.sync.dma_start(kvi[:, :, R:2 * R], kvl[:, :, R:2 * R])
                nc.gpsimd.collective_compute(
                    "AllGather", ALU.bypass, replica_groups=RG,
                    ins=[kvi.opt()], outs=[kv2.ap()])

            # stage gathered K/V into SBUF, one DMA per rank (inner runs big).
            # K pieces first: scores(h0) chases them.
            ktg = actp.tile([128, NC_, HEADS, R], fp8)
            vgl = actp.tile([128, NC_, HEADS, 4, HD], fp8)
            for r in range(NC_):
                src_r = rank if sim1 else r
                eng = [nc.sync, nc.scalar][r % 2]
                eng.dma_start(ktg[:, r, :, :], kv2[src_r, :, :, 0:R])
            for r in range(NC_):
                src_r = rank if sim1 else r
                eng = [nc.sync, nc.scalar][r % 2]
                eng.dma_start(
                    vgl[:, r, :, :, :],
                    kv2[src_r, :, :, R:2 * R].rearrange(
                        "p h (a b) -> p h a b", a=4))

            # ================= attention =================
            zb = wts.tile([128, HEADS], f32)
            nc.vector.tensor_scalar_mul(zb[:], wft[:, WF_BV:WF_BV + HEADS],
                                        float(R))
            zf = actp.tile([128, HEADS], f32)
            junk = wts.tile([128, R], f32)
            pts = {}
            cds = {}

            def pass2_den(hh):
                ps_cd = pscd.tile([128, 1024], f32, tag="d")
                cds[hh] = ps_cd
                pth = pts[hh]
                for pc in range(KB // 2):
                    nc.tensor.matmul(ps_cd[:, R:2 * R], ones2[:, :, :],
                                     pth[:, 2 * pc:2 * pc + 2, :],
                                     start=(pc == 0), stop=(pc == KB // 2 - 1),
                                     perf_mode=PM.DoubleRow,
                                     skip_group_check=True)

            def pass2_ctx(hh):
                ps_cd = cds[hh]
                pth = pts[hh]
                for pc in range(KB // 2):
                    r, mm = (2 * pc) // 4, (2 * pc) % 4
                    nc.tensor.matmul(ps_cd[:, 0:R],
                                     vgl[:, r, hh, mm:mm + 2, :],
                                     pth[:, 2 * pc:2 * pc + 2, :],
                                     start=(pc == 0), stop=(pc == KB // 2 - 1),
                                     perf_mode=PM.DoubleRow,
                                     skip_group_check=True)

            zb16 = actp.tile([128, HEADS], f32)

            def tail(hh):
                ps_cd = cds[hh]
                rbc = stg.tile([128, R], f32, tag="rbc")
                nc.vector.reciprocal(rbc[:], ps_cd[:, R:2 * R])
                zr = stg.tile([128, 1], f32, tag="zr")
                nc.vector.scalar_tensor_tensor(
                    junk[:], ps_cd[:, 0:R], 0.0, rbc[:],
                    ALU.bypass, ALU.mult, accum_out=zr[:])
                nc.gpsimd.tensor_add(zf[:, hh:hh + 1], zr[:], zb[:, hh:hh + 1])
                # fold this head's context into the out_proj accumulation
                nc.gpsimd.tensor_scalar_mul(zb16[:, hh:hh + 1],
                                            zf[:, hh:hh + 1], 1.0 / float(N))

            # exp chunk routing: per head 16 chunks of [128,1024];
            # 9 -> Act (exp from PSUM), 7 -> DVE copy + Pool 2^x
            POOL_SETS = [{2, 5, 7, 10, 12, 15}, {1, 3, 5, 8, 10, 12, 14}]

            def score_pair(h, pc):
                pt = pts[h]
                mc0, mc1 = 2 * pc, 2 * pc + 1
                ps_sc = psc.tile([128, 1024], f32, tag="c")
                nc.tensor.matmul(
                    ps_sc[:, 0:R],
                    ktg[:, mc0 // 4, h,
                        (mc0 % 4) * 128:(mc0 % 4) * 128 + 128],
                    qt[:, h, :], start=True, stop=True,
                    skip_group_check=True)
                nc.tensor.matmul(
                    ps_sc[:, R:2 * R],
                    ktg[:, mc1 // 4, h,
                        (mc1 % 4) * 128:(mc1 % 4) * 128 + 128],
                    qt[:, h, :], start=True, stop=True,
                    skip_group_check=True)
                if pc in POOL_SETS[h % 2]:
                    st = sce.tile([128, 1024], f32, tag="st")
                    nc.vector.tensor_copy(st[:, :], ps_sc[:, 0:1024])
                    nc.gpsimd.tensor_tensor(pt[:, 2 * pc:2 * pc + 2, :],
                                            two1k[:, :], st[:, :], ALU.pow)
                else:
                    nc.scalar.activation(pt[:, 2 * pc:2 * pc + 2, :],
                                         ps_sc[:, 0:1024], AF.Exp,
                                         scale=LN2 * SCL)

            # interleave prev head's den/ctx DR groups between score chunks
            # so the exp engines never starve at head boundaries
            for h in range(HEADS):
                pt_h = ptp.tile([128, KB, R], fp8, tag="pt")
                pts[h] = pt_h
                for pc in range(8):
                    score_pair(h, pc)
                if h >= 1:
                    pass2_den(h - 1)
                for pc in range(8, 12):
                    score_pair(h, pc)
                if h >= 1:
                    pass2_ctx(h - 1)
                for pc in range(12, KB // 2):
                    score_pair(h, pc)
                if h >= 1:
                    tail(h - 1)
            pass2_den(HEADS - 1)
            pass2_ctx(HEADS - 1)
            tail(HEADS - 1)

            # ========== folded out_proj@fc: out = (sum zf/N) @ Wfold ==========
            ps_fc = psc.tile([128, 1024], f32, tag="c")
            for h in range(HEADS):
                nc.tensor.matmul(ps_fc[0:1, 0:2], zb16[:, h:h + 1],
                                 wft[:, WF_FOLD + 2 * h:WF_FOLD + 2 * h + 2],
                                 start=(h == 0), stop=(h == HEADS - 1),
                                 skip_group_check=True)
            ores = stg.tile([1, 2], f32, tag="ores")
            nc.vector.tensor_add(ores[:], ps_fc[0:1, 0:2],
                                 wft[0:1, WF_FCB:WF_FCB + 2])
            nc.sync.dma_start(out_d[:, :], ores[:])

    nc.compile()
    return nc


def _pack_inputs(inputs):
    """Pack full inputs into per-core shards + replicated weight blocks."""
    fp8 = ml_dtypes.float8_e4m3
    bf16 = ml_dtypes.bfloat16

    adj = np.ascontiguousarray(inputs["adj_matrix"], dtype=np.float32)
    x = np.ascontiguousarray(inputs["node_features"], dtype=np.float32)
    W1 = np.asarray(inputs["W1"], np.float32)
    b1 = np.asarray(inputs["b1"], np.float32)
    W2 = np.asarray(inputs["W2"], np.float32)
    b2 = np.asarray(inputs["b2"], np.float32)
    ipw = np.asarray(inputs["in_proj_w"], np.float32)
    ipb = np.asarray(inputs["in_proj_b"], np.float32)
    wo = np.asarray(inputs["out_proj_w"], np.float32)
    bo = np.asarray(inputs["out_proj_b"], np.float32)
    fcw = np.asarray(inputs["fc_w"], np.float32)
    fcb = np.asarray(inputs["fc_b"], np.float32)

    wb = np.zeros((128, WB_COLS), np.float32)
    wb[:, WB_W1:WB_W1 + G1] = W1
    wb[:, WB_W2:WB_W2 + G2] = W2
    wb = wb.astype(bf16)

    # fp8 QKV weights: [128, c-tile, 3*G2] (q | k | v)
    w8 = np.zeros((128, ET, 3 * G2), np.float32)
    for c in range(ET):
        w8[:, c, :] = ipw[c * 128:(c + 1) * 128, :]
    w8 = w8.astype(fp8)

    wf = np.zeros((128, WF_COLS), np.float32)
    wf[:, WF_B1X4:WF_B1X4 + 4 * G1] = np.tile(b1, 4)[None, :]
    wf[:, WF_B2:WF_B2 + ET] = b2.reshape(ET, 128).T
    wf[:, WF_BQ:WF_BQ + HEADS] = ipb[0:G2].reshape(HEADS, HD).T
    wf[:, WF_BK:WF_BK + HEADS] = ipb[G2:2 * G2].reshape(HEADS, HD).T
    wf[:, WF_BV:WF_BV + HEADS] = ipb[2 * G2:3 * G2].reshape(HEADS, HD).T
    wfold = (wo.astype(np.float64) @ fcw.astype(np.float64)).astype(np.float32)
    wf[:, WF_FOLD:WF_FOLD + 2 * HEADS] = wfold.reshape(HEADS, HD, 2) \
        .transpose(1, 0, 2).reshape(HD, 2 * HEADS)
    wf[0, WF_FCB:WF_FCB + 2] = (bo.astype(np.float64) @ fcw.astype(np.float64)
                                + fcb).astype(np.float32) / NC_

    xp = np.ascontiguousarray(
        x.reshape(KB, 128, F_IN).transpose(1, 0, 2)).astype(fp8)

    reps = {"wb": wb, "w8": w8, "wf": wf, "xp": xp}
    in_maps = []
    idx = np.arange(R)
    for r in range(NC_):
        cols = np.ascontiguousarray(adj[:, r * R:(r + 1) * R])
        cols[r * R + idx, idx] += 1.0   # A + I, this core's diagonal block
        adjp = np.ascontiguousarray(
            cols.reshape(KB, 128, R).transpose(1, 0, 2)).astype(fp8)
        in_maps.append({"adjp": adjp, **reps})
    return in_maps


def kernel(**inputs):
    from concourse.bass_utils import run_bass_kernel_spmd

    if "nc" not in _cache:
        _cache["nc"] = _build()
    nc = _cache["nc"]

    in_maps = _pack_inputs(inputs)
    res = run_bass_kernel_spmd(nc, in_maps, core_ids=list(range(NC_)))
    out = np.zeros(2, dtype=np.float64)
    for r in range(NC_):
        out += res.results[r]["outp"].reshape(2).astype(np.float64)
    return out.astype(np.float32)


# revision 29
# speedup vs baseline: 1.4045x; 1.0359x over previous
"""Trainium2 Bass kernel for GCN(x2) + MHA + mean + FC, sharded over 8 NeuronCores.

Sharding: 1D row partition of the 4096 nodes (512 rows/core). Each core holds
the column slice adj_hat[:, r*512:(r+1)*512] of the symmetric A+I (equal to its
row block transposed), all of x, and replicated weights. Cross-core exchanges
(on-device AllGather): degree vector, GCN1 output (dinv-prescaled fp8), K, V.

v2 changes over the 150us baseline:
- Softmax exp split across THREE engines: Act computes exp(s*ln2) straight
  from PSUM; DVE stages PSUM->SBUF copies; Pool computes 2^s via the pow ALU
  op from SBUF (Pool has no PSUM port). The 1/sqrt(hd)*log2(e) score scale is
  folded into qt at bias time, so probs = 2^score on every engine. This turns
  the attention phase from Act-bound (68us) into PE-bound (~41us).
- dinv = exp(-0.5*ln(deg)) on Act: ln/exp/relu/copy share ONE activation
  table, so the kernel performs a single table load, prewarmed at t=0.
- x shipped fp8; x1 AllGather carries dinv-prescaled fp8 (the GCN2-side
  rescale loop disappears; sender scale comes from a tiny transposed DMA of
  the core's own dinv, so the SPMD program stays rank-free); QKV projections
  run fp8 DoubleRow from x2t8.
- Adjacency DMA in 4 chunks with the degree matmul group chasing it; bulk
  weight loads are queued behind the degree roundtrip so the (single) DMA
  device is free when the latency-critical transfers arrive.
- K/V gathers staged per-rank with >=512B inner runs; no DMA issue on Pool.
Host does only slicing/packing (shard) and an 8-way sum of [2]-vector partials.
"""
import sys
sys.path.insert(0, "/opt/trn_rl_repo")
import numpy as np
import ml_dtypes

N = 4096
NC_ = 8
R = N // NC_          # 512 rows per core
KB = N // 128         # 32 node chunks
F_IN = 128
G1 = 128
G2 = 512
HEADS = 4
HD = G2 // HEADS      # 128
ET = G2 // 128        # 4 tiles of the 512-dim embedding

LN2 = float(np.log(2.0))
SCL = float(np.log2(np.e)) / float(np.sqrt(HD))  # probs = 2^(q.k) after fold

# wf (f32 misc pack) column offsets
WF_B1X4 = 0        # [128,512] b1 tiled 4x (broadcast over partitions)
WF_B2 = 512        # [128,4]
WF_BQ = 516        # [128,4]
WF_BK = 520        # [128,4]
WF_BV = 524        # [128,4]
WF_FOLD = 528      # [128,8] (Wo @ fc_w) packed per head
WF_FCB = 536       # [1,2] (bo @ fc_w + fc_b)/8 at partition 0
WF_COLS = 538

# wb (bf16 pack) column offsets
WB_W1 = 0            # [128,128]
WB_W2 = 128          # [128,512]
WB_COLS = 640

_cache = {}


def _build(sim1=False, rank=0):
    from concourse import bass, bacc, tile, mybir

    f32 = mybir.dt.float32
    bf16 = mybir.dt.bfloat16
    fp8 = mybir.dt.float8e4
    AF = mybir.ActivationFunctionType
    ALU = mybir.AluOpType
    PM = mybir.MatmulPerfMode
    AX = mybir.AxisListType
    from concourse.masks import make_identity

    nc = bacc.Bacc("TRN2", target_bir_lowering=False, debug=False,
                   num_devices=1 if sim1 else NC_)

    # ---- kernel I/O (per-core shards supplied via in_maps) ----
    adj_d = nc.dram_tensor("adjp", [128, KB, R], fp8, kind="ExternalInput")
    x_d = nc.dram_tensor("xp", [128, KB, F_IN], fp8, kind="ExternalInput")
    wb_d = nc.dram_tensor("wb", [128, WB_COLS], bf16, kind="ExternalInput")
    w8_d = nc.dram_tensor("w8", [128, ET, 3 * G2], fp8, kind="ExternalInput")
    wf_d = nc.dram_tensor("wf", [128, WF_COLS], f32, kind="ExternalInput")
    out_d = nc.dram_tensor("outp", [1, 2], f32, kind="ExternalOutput")

    dg_out = nc.dram_tensor("dg_out", [KB, 128], f32, kind="Internal",
                            addr_space="Shared")
    x1g = nc.dram_tensor("x1g", [NC_, 128, ET, G1], fp8, kind="Internal",
                         addr_space="Shared")
    kv2 = nc.dram_tensor("kv2", [NC_, 128, HEADS, 2 * R], fp8, kind="Internal",
                         addr_space="Shared")

    RG = [list(range(NC_))]

    with tile.TileContext(nc) as tc:
        with tc.tile_pool(name="wts", bufs=1) as wts, \
             tc.tile_pool(name="act", bufs=1) as actp, \
             tc.tile_pool(name="stg", bufs=2) as stg, \
             tc.tile_pool(name="sce", bufs=3) as sce, \
             tc.tile_pool(name="ptp", bufs=2) as ptp, \
             tc.tile_pool(name="psc", bufs=3, space="PSUM") as psc, \
             tc.tile_pool(name="pscd", bufs=1, space="PSUM") as pscd, \
             tc.tile_pool(name="dram", bufs=1, space="DRAM") as drp:

            # ================= constants + act-table prewarm =================
            ones2 = wts.tile([128, 2, 128], fp8)
            nc.vector.memset(ones2[:, :, :], 1.0)
            ident = wts.tile([32, 32], f32)
            make_identity(nc, ident[:, :])
            two1k = wts.tile([128, 1024], f32)
            nc.gpsimd.memset(two1k[:, :], float(2.0 ** SCL))
            # prewarm the (ln,exp,relu,copy) table while the adjacency loads
            warm = wts.tile([1, 1], f32)
            nc.vector.memset(warm[:, :], 1.0)
            wsc = stg.tile([1, 1], f32, tag="warm")
            nc.scalar.activation(wsc[:, :], warm[:, :], AF.Sqrt)

            # ---- input loads. adj first (degree chases it); x on the scalar
            # queue; bulk weights queued on sync BEHIND the degree roundtrip
            # so the DMA device is free for the latency-critical hops.
            adj8 = wts.tile([128, KB, R], fp8)
            for i in range(4):
                nc.sync.dma_start(adj8[:, 8 * i:8 * i + 8, :],
                                  adj_d[:, 8 * i:8 * i + 8, :])
            x8 = wts.tile([128, KB, F_IN], fp8)
            for i in range(2):
                nc.sync.dma_start(x8[:, 16 * i:16 * i + 16, :],
                                  x_d[:, 16 * i:16 * i + 16, :])
            wft = wts.tile([128, WF_COLS], f32)
            nc.sync.dma_start(wft[:], wf_d[:, :])

            # ================= degree (PE chases the adjacency DMA) ========
            ps_deg = psc.tile([128, 1024], f32, tag="c")
            for c in range(KB // 2):
                nc.tensor.matmul(ps_deg[:, 0:R], ones2[:, :, :],
                                 adj8[:, 2 * c:2 * c + 2, :],
                                 start=(c == 0), stop=(c == KB // 2 - 1),
                                 perf_mode=PM.DoubleRow)
            # dinv = 1/sqrt(deg); deg >= 1 always (A+I)
            dsq = stg.tile([128, R], f32, tag="dsq")
            nc.scalar.activation(dsq[:], ps_deg[:, 0:R], AF.Sqrt)
            dbc = wts.tile([128, R], f32)
            nc.vector.reciprocal(dbc[:], dsq[:])

            # ================= degree AllGather -> dcol =================
            if sim1:
                nc.sync.dma_start(dg_out[4 * rank:4 * rank + 4, :].flatten(),
                                  dbc[0:1, :].flatten())
                down_src = dg_out[4 * rank:4 * rank + 4, :].flatten()
            else:
                dg_in = drp.tile([1, R], f32, tag="dgin")
                nc.sync.dma_start(dg_in[:], dbc[0:1, :])
                nc.gpsimd.collective_compute(
                    "AllGather", ALU.bypass, replica_groups=RG,
                    ins=[dg_in.opt()], outs=[dg_out.ap()])
                down_src = dg_in[0, :]
            dg_sb = stg.tile([KB, 128], f32, tag="dgsb")
            nc.sync.dma_start(dg_sb[:, :], dg_out[:, :])
            # own dinv transposed to [node%128, mt] for the x1 sender scale
            down = wts.tile([128, ET], f32)
            nc.sync.dma_start(down[:, :],
                              down_src.rearrange("(mt p) -> p mt", p=128))
            # bulk weights now (device free until x1 AG)
            wbt = wts.tile([128, WB_COLS], bf16)
            nc.sync.dma_start(wbt[:], wb_d[:, :])
            w8t = wts.tile([128, ET, 3 * G2], fp8)
            nc.sync.dma_start(w8t[:, :, :], w8_d[:, :, :])

            ps_t = psc.tile([128, 1024], f32, tag="c")
            nc.tensor.transpose(ps_t[:, 0:KB], dg_sb[:, :], ident[:, :])
            dcol = wts.tile([128, KB], f32)
            nc.vector.tensor_copy(dcol[:], ps_t[:, 0:KB])

            # ================= GCN1 =================
            # xs8[:,kb,:] = x8[:,kb,:] * dinv[node]; 3-way engine split
            xs8 = actp.tile([128, KB, F_IN], fp8)
            for kb in range(KB):
                e = kb % 3
                if e == 0:
                    nc.vector.tensor_scalar_mul(xs8[:, kb, :], x8[:, kb, :],
                                                dcol[:, kb:kb + 1])
                elif e == 1:
                    nc.gpsimd.tensor_scalar_mul(xs8[:, kb, :], x8[:, kb, :],
                                                dcol[:, kb:kb + 1])
                else:
                    nc.scalar.activation(xs8[:, kb, :], x8[:, kb, :],
                                         AF.Copy, scale=dcol[:, kb:kb + 1])
            ps_w1 = pscd.tile([128, 1024], f32, tag="d")
            for i in range(40):
                nc.tensor.matmul(ps_w1[0:1, 0:1], dcol[:, 0:1], dcol[:, 0:1],
                                 start=True, stop=True, skip_group_check=True)
            ps_s1 = psc.tile([128, 1024], f32, tag="c")
            for c in range(KB // 2):
                nc.tensor.matmul(ps_s1[:, 0:R], xs8[:, 2 * c:2 * c + 2, :],
                                 adj8[:, 2 * c:2 * c + 2, :],
                                 start=(c == 0), stop=(c == KB // 2 - 1),
                                 perf_mode=PM.DoubleRow)
            s1t = actp.tile([128, R], bf16)
            nc.vector.tensor_mul(s1t[:], ps_s1[:, 0:R], dbc[:])

            # W1 + bias, then relu with own-dinv prescale -> fp8 for the AG
            ps_x1 = psc.tile([128, 1024], f32, tag="c")
            for mt in range(4):
                nc.tensor.matmul(ps_x1[:, mt * G1:(mt + 1) * G1],
                                 s1t[:, mt * 128:(mt + 1) * 128],
                                 wbt[:, WB_W1:WB_W1 + G1],
                                 start=True, stop=True, skip_group_check=True)
            x1b = stg.tile([128, 4 * G1], f32, tag="x1b")
            x1s = actp.tile([128, ET, G1], fp8)
            for mt in range(4):
                nc.vector.tensor_add(x1b[:, mt * G1:(mt + 1) * G1],
                                     ps_x1[:, mt * G1:(mt + 1) * G1],
                                     wft[:, WF_B1X4 + mt * G1:WF_B1X4 + (mt + 1) * G1])
                # relu(x+b)*d == relu((x+b)*d) since d > 0
                nc.scalar.activation(x1s[:, mt, :],
                                     x1b[:, mt * G1:(mt + 1) * G1], AF.Relu,
                                     scale=down[:, mt:mt + 1])
            nc.scalar.activation(wsc[:, :], x1s[0:1, ET - 1, 0:1], AF.Exp)
            if sim1:
                nc.sync.dma_start(x1g[rank, :, 0:2, :], x1s[:, 0:2, :])
                nc.sync.dma_start(x1g[rank, :, 2:4, :], x1s[:, 2:4, :])
            else:
                x1_in = drp.tile([128, ET, G1], fp8, tag="x1in")
                nc.sync.dma_start(x1_in[:, 0:2, :], x1s[:, 0:2, :])
                nc.sync.dma_start(x1_in[:, 2:4, :], x1s[:, 2:4, :])
                nc.gpsimd.collective_compute(
                    "AllGather", ALU.bypass, replica_groups=RG,
                    ins=[x1_in.opt()], outs=[x1g.ap()])

            # ================= GCN2 =================
            x1a = actp.tile([128, NC_, ET, G1], fp8)
            if sim1:
                nc.sync.dma_start(
                    x1a[:, 0:4, :, :],
                    x1g[rank, :, :, :].unsqueeze(1).broadcast_to(
                        [128, 4, ET, G1]))
                nc.sync.dma_start(
                    x1a[:, 4:8, :, :],
                    x1g[rank, :, :, :].unsqueeze(1).broadcast_to(
                        [128, 4, ET, G1]))
            else:
                nc.sync.dma_start(x1a[:, 0:4, :, :],
                                  x1g[0:4, :, :, :].transpose([1, 0, 2, 3]))
                nc.sync.dma_start(x1a[:, 4:8, :, :],
                                  x1g[4:8, :, :, :].transpose([1, 0, 2, 3]))
            ps_w = pscd.tile([128, 1024], f32, tag="d")
            for i in range(40):
                nc.tensor.matmul(ps_w[0:1, 0:2], x1a[:, 0, 0:1, 0:1],
                                 x1a[:, 0, 0:1, 0:2], start=True, stop=True,
                                 skip_group_check=True)
            ps_s2 = psc.tile([128, 1024], f32, tag="c")
            for c in range(KB // 2):
                r, mm = (2 * c) // 4, (2 * c) % 4
                nc.tensor.matmul(ps_s2[:, 0:R], x1a[:, r, mm:mm + 2, :],
                                 adj8[:, 2 * c:2 * c + 2, :],
                                 start=(c == 0), stop=(c == KB // 2 - 1),
                                 perf_mode=PM.DoubleRow)
            s2t = actp.tile([128, R], bf16)
            nc.vector.tensor_mul(s2t[:], ps_s2[:, 0:R], dbc[:])

            # W2: x2t8[e-tile, node] fp8 (+bias), feeds QKV via DoubleRow
            x2t8 = actp.tile([128, ET, R], fp8)
            for eo in range(2):
                ps_x2 = psc.tile([128, 1024], f32, tag="c")
                for ei in range(2):
                    et = 2 * eo + ei
                    nc.tensor.matmul(ps_x2[:, ei * R:(ei + 1) * R],
                                     wbt[:, WB_W2 + et * 128:WB_W2 + (et + 1) * 128],
                                     s2t[:], start=True, stop=True,
                                     skip_group_check=True)
                for ei in range(2):
                    et = 2 * eo + ei
                    if ei == 0:
                        nc.vector.tensor_scalar_add(
                            x2t8[:, et, :], ps_x2[:, ei * R:(ei + 1) * R],
                            wft[:, WF_B2 + et:WF_B2 + et + 1])
                    else:
                        nc.scalar.activation(
                            x2t8[:, et, :], ps_x2[:, ei * R:(ei + 1) * R],
                            AF.Identity,
                            bias=wft[:, WF_B2 + et:WF_B2 + et + 1])

            # ================= QKV (fp8 DoubleRow) =================
            qt = actp.tile([128, HEADS, R], fp8)
            kvl = actp.tile([128, HEADS, 2 * R], fp8)
            for hp in range(2):
                # K heads pair-wise first so the K AllGather starts early
                ps_qk = psc.tile([128, 1024], f32, tag="c")
                for hh in range(2):
                    h = 2 * hp + hh
                    for cp in range(2):
                        nc.tensor.matmul(
                            ps_qk[:, hh * R:(hh + 1) * R],
                            w8t[:, 2 * cp:2 * cp + 2, G2 + h * 128:G2 + (h + 1) * 128],
                            x2t8[:, 2 * cp:2 * cp + 2, :],
                            start=(cp == 0), stop=(cp == 1),
                            perf_mode=PM.DoubleRow, skip_group_check=True)
                for hh in range(2):
                    h = 2 * hp + hh
                    if hh == 0:
                        nc.vector.tensor_scalar_add(
                            kvl[:, h, 0:R], ps_qk[:, hh * R:(hh + 1) * R],
                            wft[:, WF_BK + h:WF_BK + h + 1])
                    else:
                        nc.scalar.activation(
                            kvl[:, h, 0:R], ps_qk[:, hh * R:(hh + 1) * R],
                            AF.Identity,
                            bias=wft[:, WF_BK + h:WF_BK + h + 1])

            # export the K half as soon as K heads are done; V follows
            if sim1:
                nc.sync.dma_start(kv2[rank, :, :, 0:R], kvl[:, :, 0:R])
            else:
                kvi = drp.tile([128, HEADS, 2 * R], fp8, tag="kvi")
                nc.sync.dma_start(kvi[:, :, 0:R], kvl[:, :, 0:R])

            for hp in range(2):
                ps_qk = psc.tile([128, 1024], f32, tag="c")
                for hh in range(2):
                    h = 2 * hp + hh
                    for cp in range(2):
                        nc.tensor.matmul(
                            ps_qk[:, hh * R:(hh + 1) * R],
                            w8t[:, 2 * cp:2 * cp + 2, h * 128:(h + 1) * 128],
                            x2t8[:, 2 * cp:2 * cp + 2, :],
                            start=(cp == 0), stop=(cp == 1),
                            perf_mode=PM.DoubleRow, skip_group_check=True)
                for hh in range(2):
                    h = 2 * hp + hh
                    # plain q + bq; the 1/sqrt(hd) softmax scale is folded
                    # into the exp bases (act scale / pool pow base)
                    if hh == 0:
                        nc.vector.tensor_scalar_add(
                            qt[:, h, :], ps_qk[:, hh * R:(hh + 1) * R],
                            wft[:, WF_BQ + h:WF_BQ + h + 1])
                    else:
                        nc.scalar.activation(
                            qt[:, h, :], ps_qk[:, hh * R:(hh + 1) * R],
                            AF.Identity,
                            bias=wft[:, WF_BQ + h:WF_BQ + h + 1])

            for h in range(HEADS):
                ps_v = psc.tile([128, 1024], f32, tag="c")
                for mt in range(4):
                    for cp in range(2):
                        nc.tensor.matmul(
                            ps_v[:, mt * HD:(mt + 1) * HD],
                            x2t8[:, 2 * cp:2 * cp + 2, mt * 128:(mt + 1) * 128],
                            w8t[:, 2 * cp:2 * cp + 2,
                                2 * G2 + h * HD:2 * G2 + (h + 1) * HD],
                            start=(cp == 0), stop=(cp == 1),
                            perf_mode=PM.DoubleRow, skip_group_check=True)
                if h % 2 == 0:
                    nc.vector.tensor_copy(kvl[:, h, R:2 * R], ps_v[:, 0:4 * HD])
                else:
                    nc.scalar.activation(kvl[:, h, R:2 * R], ps_v[:, 0:4 * HD],
                                         AF.Copy)
            if sim1:
                nc.sync.dma_start(kv2[rank, :, :, R:2 * R], kvl[:, :, R:2 * R])
            else:
                nc.sync.dma_start(kvi[:, :, R:2 * R], kvl[:, :, R:2 * R])
                nc.gpsimd.collective_compute(
                    "AllGather", ALU.bypass, replica_groups=RG,
                    ins=[kvi.opt()], outs=[kv2.ap()])

            # stage gathered K/V into SBUF, one DMA per rank (inner runs big).
            # K pieces first: scores(h0) chases them.
            ktg = actp.tile([128, NC_, HEADS, R], fp8)
            vgl = actp.tile([128, NC_, HEADS, 4, HD], fp8)
            for r in range(NC_):
                src_r = rank if sim1 else r
                eng = [nc.sync, nc.scalar][r % 2]
                eng.dma_start(ktg[:, r, :, :], kv2[src_r, :, :, 0:R])
            for r in range(NC_):
                src_r = rank if sim1 else r
                eng = [nc.sync, nc.scalar][r % 2]
                eng.dma_start(
                    vgl[:, r, :, :, :],
                    kv2[src_r, :, :, R:2 * R].rearrange(
                        "p h (a b) -> p h a b", a=4))

            # ================= attention =================
            zb = wts.tile([128, HEADS], f32)
            nc.vector.tensor_scalar_mul(zb[:], wft[:, WF_BV:WF_BV + HEADS],
                                        float(R))
            zf = actp.tile([128, HEADS], f32)
            junk = wts.tile([128, R], f32)
            pts = {}
            cds = {}

            def pass2_den(hh):
                ps_cd = pscd.tile([128, 1024], f32, tag="d")
                cds[hh] = ps_cd
                pth = pts[hh]
                for pc in range(KB // 2):
                    nc.tensor.matmul(ps_cd[:, R:2 * R], ones2[:, :, :],
                                     pth[:, 2 * pc:2 * pc + 2, :],
                                     start=(pc == 0), stop=(pc == KB // 2 - 1),
                                     perf_mode=PM.DoubleRow,
                                     skip_group_check=True)

            def pass2_ctx(hh):
                ps_cd = cds[hh]
                pth = pts[hh]
                for pc in range(KB // 2):
                    r, mm = (2 * pc) // 4, (2 * pc) % 4
                    nc.tensor.matmul(ps_cd[:, 0:R],
                                     vgl[:, r, hh, mm:mm + 2, :],
                                     pth[:, 2 * pc:2 * pc + 2, :],
                                     start=(pc == 0), stop=(pc == KB // 2 - 1),
                                     perf_mode=PM.DoubleRow,
                                     skip_group_check=True)

            zb16 = actp.tile([128, HEADS], f32)

            def tail(hh):
                ps_cd = cds[hh]
                rbc = stg.tile([128, R], f32, tag="rbc")
                nc.vector.reciprocal(rbc[:], ps_cd[:, R:2 * R])
                zr = stg.tile([128, 1], f32, tag="zr")
                nc.vector.scalar_tensor_tensor(
                    junk[:], ps_cd[:, 0:R], 0.0, rbc[:],
                    ALU.bypass, ALU.mult, accum_out=zr[:])
                nc.gpsimd.tensor_add(zf[:, hh:hh + 1], zr[:], zb[:, hh:hh + 1])
                # fold this head's context into the out_proj accumulation
                nc.gpsimd.tensor_scalar_mul(zb16[:, hh:hh + 1],
                                            zf[:, hh:hh + 1], 1.0 / float(N))

            # exp chunk routing: per head 16 chunks of [128,1024];
            # 9 -> Act (exp from PSUM), 7 -> DVE copy + Pool 2^x
            POOL_SETS = [{2, 5, 7, 10, 12, 15}, {1, 3, 5, 8, 10, 12, 14}]

            def score_pair(h, pc):
                pt = pts[h]
                mc0, mc1 = 2 * pc, 2 * pc + 1
                ps_sc = psc.tile([128, 1024], f32, tag="c")
                nc.tensor.matmul(
                    ps_sc[:, 0:R],
                    ktg[:, mc0 // 4, h,
                        (mc0 % 4) * 128:(mc0 % 4) * 128 + 128],
                    qt[:, h, :], start=True, stop=True,
                    skip_group_check=True)
                nc.tensor.matmul(
                    ps_sc[:, R:2 * R],
                    ktg[:, mc1 // 4, h,
                        (mc1 % 4) * 128:(mc1 % 4) * 128 + 128],
                    qt[:, h, :], start=True, stop=True,
                    skip_group_check=True)
                if pc in POOL_SETS[h % 2]:
                    st = sce.tile([128, 1024], f32, tag="st")
                    nc.vector.tensor_copy(st[:, :], ps_sc[:, 0:1024])
                    nc.gpsimd.tensor_tensor(pt[:, 2 * pc:2 * pc + 2, :],
                                            two1k[:, :], st[:, :], ALU.pow)
                else:
                    nc.scalar.activation(pt[:, 2 * pc:2 * pc + 2, :],
                                         ps_sc[:, 0:1024], AF.Exp,
                                         scale=LN2 * SCL)

            # interleave prev head's den/ctx DR groups between score chunks
            # so the exp engines never starve at head boundaries
            for h in range(HEADS):
                pt_h = ptp.tile([128, KB, R], fp8, tag="pt")
                pts[h] = pt_h
                for pc in range(8):
                    score_pair(h, pc)
                if h >= 1:
                    pass2_den(h - 1)
                for pc in range(8, 12):
                    score_pair(h, pc)
                if h >= 1:
                    pass2_ctx(h - 1)
                for pc in range(12, KB // 2):
                    score_pair(h, pc)
                if h >= 1:
                    tail(h - 1)
            pass2_den(HEADS - 1)
            pass2_ctx(HEADS - 1)
            tail(HEADS - 1)

            # ========== folded out_proj@fc: out = (sum zf/N) @ Wfold ==========
            ps_fc = psc.tile([128, 1024], f32, tag="c")
            for h in range(HEADS):
                nc.tensor.matmul(ps_fc[0:1, 0:2], zb16[:, h:h + 1],
                                 wft[:, WF_FOLD + 2 * h:WF_FOLD + 2 * h + 2],
                                 start=(h == 0), stop=(h == HEADS - 1),
                                 skip_group_check=True)
            ores = stg.tile([1, 2], f32, tag="ores")
            nc.vector.tensor_add(ores[:], ps_fc[0:1, 0:2],
                                 wft[0:1, WF_FCB:WF_FCB + 2])
            nc.sync.dma_start(out_d[:, :], ores[:])

    nc.compile()
    return nc


def _pack_inputs(inputs):
    """Pack full inputs into per-core shards + replicated weight blocks."""
    fp8 = ml_dtypes.float8_e4m3
    bf16 = ml_dtypes.bfloat16

    adj = np.ascontiguousarray(inputs["adj_matrix"], dtype=np.float32)
    x = np.ascontiguousarray(inputs["node_features"], dtype=np.float32)
    W1 = np.asarray(inputs["W1"], np.float32)
    b1 = np.asarray(inputs["b1"], np.float32)
    W2 = np.asarray(inputs["W2"], np.float32)
    b2 = np.asarray(inputs["b2"], np.float32)
    ipw = np.asarray(inputs["in_proj_w"], np.float32)
    ipb = np.asarray(inputs["in_proj_b"], np.float32)
    wo = np.asarray(inputs["out_proj_w"], np.float32)
    bo = np.asarray(inputs["out_proj_b"], np.float32)
    fcw = np.asarray(inputs["fc_w"], np.float32)
    fcb = np.asarray(inputs["fc_b"], np.float32)

    wb = np.zeros((128, WB_COLS), np.float32)
    wb[:, WB_W1:WB_W1 + G1] = W1
    wb[:, WB_W2:WB_W2 + G2] = W2
    wb = wb.astype(bf16)

    # fp8 QKV weights: [128, c-tile, 3*G2] (q | k | v)
    w8 = np.zeros((128, ET, 3 * G2), np.float32)
    for c in range(ET):
        w8[:, c, :] = ipw[c * 128:(c + 1) * 128, :]
    w8 = w8.astype(fp8)

    wf = np.zeros((128, WF_COLS), np.float32)
    wf[:, WF_B1X4:WF_B1X4 + 4 * G1] = np.tile(b1, 4)[None, :]
    wf[:, WF_B2:WF_B2 + ET] = b2.reshape(ET, 128).T
    wf[:, WF_BQ:WF_BQ + HEADS] = ipb[0:G2].reshape(HEADS, HD).T
    wf[:, WF_BK:WF_BK + HEADS] = ipb[G2:2 * G2].reshape(HEADS, HD).T
    wf[:, WF_BV:WF_BV + HEADS] = ipb[2 * G2:3 * G2].reshape(HEADS, HD).T
    wfold = (wo.astype(np.float64) @ fcw.astype(np.float64)).astype(np.float32)
    wf[:, WF_FOLD:WF_FOLD + 2 * HEADS] = wfold.reshape(HEADS, HD, 2) \
        .transpose(1, 0, 2).reshape(HD, 2 * HEADS)
    wf[0, WF_FCB:WF_FCB + 2] = (bo.astype(np.float64) @ fcw.astype(np.float64)
                                + fcb).astype(np.float32) / NC_

    xp = np.ascontiguousarray(
        x.reshape(KB, 128, F_IN).transpose(1, 0, 2)).astype(fp8)

    reps = {"wb": wb, "w8": w8, "wf": wf, "xp": xp}
    in_maps = []
    idx = np.arange(R)
    for r in range(NC_):
        cols = np.ascontiguousarray(adj[:, r * R:(r + 1) * R])
        cols[r * R + idx, idx] += 1.0   # A + I, this core's diagonal block
        adjp = np.ascontiguousarray(
            cols.reshape(KB, 128, R).transpose(1, 0, 2)).astype(fp8)
        in_maps.append({"adjp": adjp, **reps})
    return in_maps


def kernel(**inputs):
    from concourse.bass_utils import run_bass_kernel_spmd

    if "nc" not in _cache:
        _cache["nc"] = _build()
    nc = _cache["nc"]

    in_maps = _pack_inputs(inputs)
    res = run_bass_kernel_spmd(nc, in_maps, core_ids=list(range(NC_)))
    out = np.zeros(2, dtype=np.float64)
    for r in range(NC_):
        out += res.results[r]["outp"].reshape(2).astype(np.float64)
    return out.astype(np.float32)


# revision 51
# speedup vs baseline: 1.4896x; 1.0606x over previous
"""Trainium2 Bass kernel for GCN(x2) + MHA + mean + FC, sharded over 8 NeuronCores.

Sharding: 1D row partition of the 4096 nodes (512 rows/core). Each core holds
the column slice adj_hat[:, r*512:(r+1)*512] of the symmetric A+I (equal to its
row block transposed), all of x, and replicated weights. Cross-core exchanges
(on-device AllGather): degree vector, GCN1 output (dinv-prescaled fp8), and a
fused K|V buffer (fp8).

Design (~101.7us TimelineSim vs 150.2us baseline):
- Softmax exp split across three engines: Act computes exp(s*ln2*scl) straight
  from PSUM (~10 of 16 chunks/head); DVE stages PSUM->SBUF copies; Pool
  computes pow(2^scl, s) from SBUF via the pow ALU op (Pool has no PSUM port).
  The 1/sqrt(hd) softmax scale is folded into the exp bases, so Q/K biases
  stay plain adds. Attention becomes PE-bound (~10.3us/head) instead of
  Act-bound (~17us/head).
- The previous head's softmax-denominator and context DoubleRow groups are
  interleaved BETWEEN score chunks (groups stay internally contiguous), so
  the exp engines never starve at head boundaries; den and ctx live in
  SEPARATE 1-bank PSUM tiles (tile-granular deps would otherwise serialize
  the reciprocal behind the ctx group), and group members accumulate
  Act-exp'd chunks first so they never wait on Pool-path straggler exps.
- All attention operands fp8: scores QK^T in fp8 (same PE speed, half the
  K-gather DMA), probs fp8, V fp8 DoubleRow; den+ctx via fp8 DoubleRow.
- dinv = 1/sqrt(deg) with the sqrt table prewarmed at t=0 and the exp table
  prewarmed mid-GCN (act-table loads are off the critical path; ln<->exp
  alternation avoided since the loader picks single-function tables).
- x shipped fp8; x1 AllGather carries dinv-prescaled fp8 (sender-side scale
  via a tiny transposed DMA of the core's own dinv keeps the SPMD program
  rank-free); QKV projections run fp8 DoubleRow from x2t8.
- K half of the fused K|V export ships as soon as K heads finish, so the
  gathered-K staging (and head-0 scores) start while V is still computing.
- Adjacency DMA in 4 chunks with the degree matmul group chasing it; bulk
  weight loads are ordered behind the latency-critical transfers (single DMA
  device); out_proj@fc folded into one [512,2] weight on the host, making the
  kernel tail 4 tiny matmuls.
- Tiny matmul bursts gated on gather outputs re-ramp the PE clock (p-state)
  after long DMA/collective idles.
Host does only slicing/packing (shard) and an 8-way sum of [2]-vector partials.
"""
import sys
sys.path.insert(0, "/opt/trn_rl_repo")
import numpy as np
import ml_dtypes

N = 4096
NC_ = 8
R = N // NC_          # 512 rows per core
KB = N // 128         # 32 node chunks
F_IN = 128
G1 = 128
G2 = 512
HEADS = 4
HD = G2 // HEADS      # 128
ET = G2 // 128        # 4 tiles of the 512-dim embedding

LN2 = float(np.log(2.0))
SCL = float(np.log2(np.e)) / float(np.sqrt(HD))  # probs = 2^(q.k) after fold

# wf (f32 misc pack) column offsets
WF_B1X4 = 0        # [128,512] b1 tiled 4x (broadcast over partitions)
WF_B2 = 512        # [128,4]
WF_BQ = 516        # [128,4]
WF_BK = 520        # [128,4]
WF_BV = 524        # [128,4]
WF_FOLD = 528      # [128,8] (Wo @ fc_w) packed per head
WF_FCB = 536       # [1,2] (bo @ fc_w + fc_b)/8 at partition 0
WF_COLS = 538

# wb (bf16 pack) column offsets
WB_W1 = 0            # [128,128]
WB_W2 = 128          # [128,512]
WB_COLS = 640

_cache = {}


def _build(sim1=False, rank=0):
    from concourse import bass, bacc, tile, mybir

    f32 = mybir.dt.float32
    bf16 = mybir.dt.bfloat16
    fp8 = mybir.dt.float8e4
    AF = mybir.ActivationFunctionType
    ALU = mybir.AluOpType
    PM = mybir.MatmulPerfMode
    from concourse.masks import make_identity

    nc = bacc.Bacc("TRN2", target_bir_lowering=False, debug=False,
                   num_devices=1 if sim1 else NC_)

    # ---- kernel I/O (per-core shards supplied via in_maps) ----
    adj_d = nc.dram_tensor("adjp", [128, KB, R], fp8, kind="ExternalInput")
    x_d = nc.dram_tensor("xp", [128, KB, F_IN], fp8, kind="ExternalInput")
    wb_d = nc.dram_tensor("wb", [128, WB_COLS], bf16, kind="ExternalInput")
    w8_d = nc.dram_tensor("w8", [128, ET, 3 * G2], fp8, kind="ExternalInput")
    wf_d = nc.dram_tensor("wf", [128, WF_COLS], f32, kind="ExternalInput")
    out_d = nc.dram_tensor("outp", [1, 2], f32, kind="ExternalOutput")

    dg_out = nc.dram_tensor("dg_out", [KB, 128], f32, kind="Internal",
                            addr_space="Shared")
    x1g = nc.dram_tensor("x1g", [NC_, 128, ET, G1], fp8, kind="Internal",
                         addr_space="Shared")
    kv2 = nc.dram_tensor("kv2", [NC_, 128, HEADS, 2 * R], fp8, kind="Internal",
                         addr_space="Shared")

    RG = [list(range(NC_))]

    with tile.TileContext(nc) as tc:
        with tc.tile_pool(name="wts", bufs=1) as wts, \
             tc.tile_pool(name="act", bufs=1) as actp, \
             tc.tile_pool(name="stg", bufs=2) as stg, \
             tc.tile_pool(name="sce", bufs=3) as sce, \
             tc.tile_pool(name="ptp", bufs=2) as ptp, \
             tc.tile_pool(name="psc", bufs=3, space="PSUM") as psc, \
             tc.tile_pool(name="psdn", bufs=1, space="PSUM") as psdn, \
             tc.tile_pool(name="psct", bufs=1, space="PSUM") as psct, \
             tc.tile_pool(name="dram", bufs=1, space="DRAM") as drp:

            # ================= constants + act-table prewarm =================
            ones2 = wts.tile([128, 2, 128], fp8)
            nc.vector.memset(ones2[:, :, :], 1.0)
            ident = wts.tile([32, 32], f32)
            make_identity(nc, ident[:, :])
            two1k = wts.tile([128, 1024], f32)
            nc.gpsimd.memset(two1k[:, :], float(2.0 ** SCL))
            # prewarm the (ln,exp,relu,copy) table while the adjacency loads
            warm = wts.tile([1, 1], f32)
            nc.vector.memset(warm[:, :], 1.0)
            wsc = stg.tile([1, 1], f32, tag="warm")
            nc.scalar.activation(wsc[:, :], warm[:, :], AF.Sqrt)

            # ---- input loads. adj first (degree chases it); x on the scalar
            # queue; bulk weights queued on sync BEHIND the degree roundtrip
            # so the DMA device is free for the latency-critical hops.
            adj8 = wts.tile([128, KB, R], fp8)
            for i in range(4):
                nc.sync.dma_start(adj8[:, 8 * i:8 * i + 8, :],
                                  adj_d[:, 8 * i:8 * i + 8, :])
            x8 = wts.tile([128, KB, F_IN], fp8)
            for i in range(2):
                nc.sync.dma_start(x8[:, 16 * i:16 * i + 16, :],
                                  x_d[:, 16 * i:16 * i + 16, :])
            wft = wts.tile([128, WF_COLS], f32)
            nc.sync.dma_start(wft[:], wf_d[:, :])

            # ================= degree (PE chases the adjacency DMA) ========
            ps_deg = psc.tile([128, 1024], f32, tag="c")
            for c in range(KB // 2):
                nc.tensor.matmul(ps_deg[:, 0:R], ones2[:, :, :],
                                 adj8[:, 2 * c:2 * c + 2, :],
                                 start=(c == 0), stop=(c == KB // 2 - 1),
                                 perf_mode=PM.DoubleRow)
            # dinv = 1/sqrt(deg); deg >= 1 always (A+I)
            dsq = stg.tile([128, R], f32, tag="dsq")
            nc.scalar.activation(dsq[:], ps_deg[:, 0:R], AF.Sqrt)
            dbc = wts.tile([128, R], f32)
            nc.vector.reciprocal(dbc[:], dsq[:])

            # ================= degree AllGather -> dcol =================
            if sim1:
                nc.sync.dma_start(dg_out[4 * rank:4 * rank + 4, :].flatten(),
                                  dbc[0:1, :].flatten())
                down_src = dg_out[4 * rank:4 * rank + 4, :].flatten()
            else:
                dg_in = drp.tile([1, R], f32, tag="dgin")
                nc.sync.dma_start(dg_in[:], dbc[0:1, :])
                nc.gpsimd.collective_compute(
                    "AllGather", ALU.bypass, replica_groups=RG,
                    ins=[dg_in.opt()], outs=[dg_out.ap()])
                down_src = dg_in[0, :]
            dg_sb = stg.tile([KB, 128], f32, tag="dgsb")
            nc.sync.dma_start(dg_sb[:, :], dg_out[:, :])
            # own dinv transposed to [node%128, mt] for the x1 sender scale
            down = wts.tile([128, ET], f32)
            nc.sync.dma_start(down[:, :],
                              down_src.rearrange("(mt p) -> p mt", p=128))
            # bulk weights now (device free until x1 AG)
            wbt = wts.tile([128, WB_COLS], bf16)
            nc.sync.dma_start(wbt[:], wb_d[:, :])
            w8t = wts.tile([128, ET, 3 * G2], fp8)
            nc.sync.dma_start(w8t[:, :, :], w8_d[:, :, :])

            ps_t = psc.tile([128, 1024], f32, tag="c")
            nc.tensor.transpose(ps_t[:, 0:KB], dg_sb[:, :], ident[:, :])
            dcol = wts.tile([128, KB], f32)
            nc.vector.tensor_copy(dcol[:], ps_t[:, 0:KB])

            # ================= GCN1 =================
            # xs8[:,kb,:] = x8[:,kb,:] * dinv[node]; 3-way engine split
            xs8 = actp.tile([128, KB, F_IN], fp8)
            for kb in range(KB):
                e = kb % 3
                if e == 0:
                    nc.vector.tensor_scalar_mul(xs8[:, kb, :], x8[:, kb, :],
                                                dcol[:, kb:kb + 1])
                elif e == 1:
                    nc.gpsimd.tensor_scalar_mul(xs8[:, kb, :], x8[:, kb, :],
                                                dcol[:, kb:kb + 1])
                else:
                    nc.scalar.activation(xs8[:, kb, :], x8[:, kb, :],
                                         AF.Copy, scale=dcol[:, kb:kb + 1])
            ps_w1 = psdn.tile([128, 512], f32, tag="d")
            for i in range(40):
                nc.tensor.matmul(ps_w1[0:1, 0:1], dcol[:, 0:1], dcol[:, 0:1],
                                 start=True, stop=True, skip_group_check=True)
            ps_s1 = psc.tile([128, 1024], f32, tag="c")
            for c in range(KB // 2):
                nc.tensor.matmul(ps_s1[:, 0:R], xs8[:, 2 * c:2 * c + 2, :],
                                 adj8[:, 2 * c:2 * c + 2, :],
                                 start=(c == 0), stop=(c == KB // 2 - 1),
                                 perf_mode=PM.DoubleRow)
            s1t = actp.tile([128, R], bf16)
            nc.vector.tensor_mul(s1t[:], ps_s1[:, 0:R], dbc[:])

            # W1 + bias, then relu with own-dinv prescale -> fp8 for the AG
            ps_x1 = psc.tile([128, 1024], f32, tag="c")
            for mt in range(4):
                nc.tensor.matmul(ps_x1[:, mt * G1:(mt + 1) * G1],
                                 s1t[:, mt * 128:(mt + 1) * 128],
                                 wbt[:, WB_W1:WB_W1 + G1],
                                 start=True, stop=True, skip_group_check=True)
            x1b = stg.tile([128, 4 * G1], f32, tag="x1b")
            x1s = actp.tile([128, ET, G1], fp8)
            for mt in range(4):
                nc.vector.tensor_add(x1b[:, mt * G1:(mt + 1) * G1],
                                     ps_x1[:, mt * G1:(mt + 1) * G1],
                                     wft[:, WF_B1X4 + mt * G1:WF_B1X4 + (mt + 1) * G1])
                # relu(x+b)*d == relu((x+b)*d) since d > 0
                nc.scalar.activation(x1s[:, mt, :],
                                     x1b[:, mt * G1:(mt + 1) * G1], AF.Relu,
                                     scale=down[:, mt:mt + 1])
            nc.scalar.activation(wsc[:, :], x1s[0:1, ET - 1, 0:1], AF.Exp)
            if sim1:
                nc.sync.dma_start(x1g[rank, :, :, :], x1s[:, :, :])
            else:
                x1_in = drp.tile([128, ET, G1], fp8, tag="x1in")
                nc.sync.dma_start(x1_in[:, :, :], x1s[:, :, :])
                nc.gpsimd.collective_compute(
                    "AllGather", ALU.bypass, replica_groups=RG,
                    ins=[x1_in.opt()], outs=[x1g.ap()])

            # ================= GCN2 =================
            x1a = actp.tile([128, NC_, ET, G1], fp8)
            if sim1:
                nc.sync.dma_start(
                    x1a[:, :, :, :],
                    x1g[rank, :, :, :].unsqueeze(1).broadcast_to(
                        [128, NC_, ET, G1]))
            else:
                nc.sync.dma_start(x1a[:, :, :, :],
                                  x1g[:, :, :, :].transpose([1, 0, 2, 3]))
            ps_w = psdn.tile([128, 512], f32, tag="d")
            for i in range(40):
                nc.tensor.matmul(ps_w[0:1, 0:2], x1a[:, 0, 0:1, 0:1],
                                 x1a[:, 0, 0:1, 0:2], start=True, stop=True,
                                 skip_group_check=True)
            ps_s2 = psc.tile([128, 1024], f32, tag="c")
            for c in range(KB // 2):
                r, mm = (2 * c) // 4, (2 * c) % 4
                nc.tensor.matmul(ps_s2[:, 0:R], x1a[:, r, mm:mm + 2, :],
                                 adj8[:, 2 * c:2 * c + 2, :],
                                 start=(c == 0), stop=(c == KB // 2 - 1),
                                 perf_mode=PM.DoubleRow)
            s2t = actp.tile([128, R], bf16)
            nc.vector.tensor_mul(s2t[:], ps_s2[:, 0:R], dbc[:])

            # W2: x2t8[e-tile, node] fp8 (+bias), feeds QKV via DoubleRow
            x2t8 = actp.tile([128, ET, R], fp8)
            for et in range(ET):
                ps_xh = psc.tile([128, 512], f32, tag="c")
                nc.tensor.matmul(ps_xh[:, 0:R],
                                 wbt[:, WB_W2 + et * 128:WB_W2 + (et + 1) * 128],
                                 s2t[:], start=True, stop=True,
                                 skip_group_check=True)
                if et % 2 == 0:
                    nc.vector.tensor_scalar_add(
                        x2t8[:, et, :], ps_xh[:, 0:R],
                        wft[:, WF_B2 + et:WF_B2 + et + 1])
                else:
                    nc.scalar.activation(
                        x2t8[:, et, :], ps_xh[:, 0:R],
                        AF.Identity,
                        bias=wft[:, WF_B2 + et:WF_B2 + et + 1])

            # ================= QKV (fp8 DoubleRow) =================
            qt = actp.tile([128, HEADS, R], fp8)
            kvl = actp.tile([128, HEADS, 2 * R], fp8)
            for hp in range(2):
                # K heads pair-wise; each half-tile releases independently
                for hh in range(2):
                    h = 2 * hp + hh
                    ps_kh = (psdn if hh == 0 else psct).tile(
                        [128, 512], f32, tag="d" if hh == 0 else "t")
                    for cp in range(2):
                        nc.tensor.matmul(
                            ps_kh[:, 0:R],
                            w8t[:, 2 * cp:2 * cp + 2, G2 + h * 128:G2 + (h + 1) * 128],
                            x2t8[:, 2 * cp:2 * cp + 2, :],
                            start=(cp == 0), stop=(cp == 1),
                            perf_mode=PM.DoubleRow, skip_group_check=True)
                    if hh == 0:
                        nc.vector.tensor_scalar_add(
                            kvl[:, h, 0:R], ps_kh[:, 0:R],
                            wft[:, WF_BK + h:WF_BK + h + 1])
                    else:
                        nc.scalar.activation(
                            kvl[:, h, 0:R], ps_kh[:, 0:R],
                            AF.Identity,
                            bias=wft[:, WF_BK + h:WF_BK + h + 1])

            # export the K half as soon as K heads are done; V follows
            if sim1:
                nc.sync.dma_start(kv2[rank, :, :, 0:R], kvl[:, :, 0:R])
            else:
                kvi = drp.tile([128, HEADS, 2 * R], fp8, tag="kvi")
                nc.sync.dma_start(kvi[:, :, 0:R], kvl[:, :, 0:R])

            for hp in range(2):
                ps_qk = psc.tile([128, 1024], f32, tag="c")
                for hh in range(2):
                    h = 2 * hp + hh
                    for cp in range(2):
                        nc.tensor.matmul(
                            ps_qk[:, hh * R:(hh + 1) * R],
                            w8t[:, 2 * cp:2 * cp + 2, h * 128:(h + 1) * 128],
                            x2t8[:, 2 * cp:2 * cp + 2, :],
                            start=(cp == 0), stop=(cp == 1),
                            perf_mode=PM.DoubleRow, skip_group_check=True)
                for hh in range(2):
                    h = 2 * hp + hh
                    # plain q + bq; the 1/sqrt(hd) softmax scale is folded
                    # into the exp bases (act scale / pool pow base)
                    if hh == 0:
                        nc.vector.tensor_scalar_add(
                            qt[:, h, :], ps_qk[:, hh * R:(hh + 1) * R],
                            wft[:, WF_BQ + h:WF_BQ + h + 1])
                    else:
                        nc.scalar.activation(
                            qt[:, h, :], ps_qk[:, hh * R:(hh + 1) * R],
                            AF.Identity,
                            bias=wft[:, WF_BQ + h:WF_BQ + h + 1])

            for h in range(HEADS):
                ps_v = psc.tile([128, 1024], f32, tag="c")
                for mt in range(4):
                    for cp in range(2):
                        nc.tensor.matmul(
                            ps_v[:, mt * HD:(mt + 1) * HD],
                            x2t8[:, 2 * cp:2 * cp + 2, mt * 128:(mt + 1) * 128],
                            w8t[:, 2 * cp:2 * cp + 2,
                                2 * G2 + h * HD:2 * G2 + (h + 1) * HD],
                            start=(cp == 0), stop=(cp == 1),
                            perf_mode=PM.DoubleRow, skip_group_check=True)
                if h % 2 == 0:
                    nc.vector.tensor_copy(kvl[:, h, R:2 * R], ps_v[:, 0:4 * HD])
                else:
                    nc.scalar.activation(kvl[:, h, R:2 * R], ps_v[:, 0:4 * HD],
                                         AF.Copy)
            if sim1:
                nc.sync.dma_start(kv2[rank, :, :, R:2 * R], kvl[:, :, R:2 * R])
            else:
                nc.sync.dma_start(kvi[:, :, R:2 * R], kvl[:, :, R:2 * R])
                nc.gpsimd.collective_compute(
                    "AllGather", ALU.bypass, replica_groups=RG,
                    ins=[kvi.opt()], outs=[kv2.ap()])

            # stage gathered K/V into SBUF, one DMA per rank (inner runs big).
            # K pieces first: scores(h0) chases them.
            ktg = actp.tile([128, NC_, HEADS, R], fp8)
            vgl = actp.tile([128, NC_, HEADS, 4, HD], fp8)
            for r in range(NC_):
                src_r = rank if sim1 else r
                eng = [nc.sync, nc.scalar][r % 2]
                eng.dma_start(ktg[:, r, :, :], kv2[src_r, :, :, 0:R])
            for r in range(NC_):
                src_r = rank if sim1 else r
                eng = [nc.sync, nc.scalar][r % 2]
                eng.dma_start(
                    vgl[:, r, :, :, :],
                    kv2[src_r, :, :, R:2 * R].rearrange(
                        "p h (a b) -> p h a b", a=4))

            # ================= attention =================
            zb = wts.tile([128, HEADS], f32)
            nc.vector.tensor_scalar_mul(zb[:], wft[:, WF_BV:WF_BV + HEADS],
                                        float(R))
            zf = actp.tile([128, HEADS], f32)
            junk = wts.tile([128, R], f32)
            pts = {}
            cds = {}
            cts = {}

            def p2_order(hh):
                # accumulate Act-exp'd chunks first; the Pool-path chunks
                # (exp lags ~2.7us behind the matmul) come last so the DR
                # group never stalls on straggler exps
                ps = POOL_SETS[hh % 2]
                return ([pc for pc in range(KB // 2) if pc not in ps]
                        + [pc for pc in range(KB // 2) if pc in ps])

            def pass2_den(hh):
                ps_dn = psdn.tile([128, 512], f32, tag="d")
                cds[hh] = ps_dn
                pth = pts[hh]
                order = p2_order(hh)
                for i, pc in enumerate(order):
                    nc.tensor.matmul(ps_dn[:, 0:R], ones2[:, :, :],
                                     pth[:, 2 * pc:2 * pc + 2, :],
                                     start=(i == 0), stop=(i == KB // 2 - 1),
                                     perf_mode=PM.DoubleRow,
                                     skip_group_check=True)

            def pass2_ctx(hh):
                ps_ct = psct.tile([128, 512], f32, tag="t")
                cts[hh] = ps_ct
                pth = pts[hh]
                order = p2_order(hh)
                for i, pc in enumerate(order):
                    r, mm = (2 * pc) // 4, (2 * pc) % 4
                    nc.tensor.matmul(ps_ct[:, 0:R],
                                     vgl[:, r, hh, mm:mm + 2, :],
                                     pth[:, 2 * pc:2 * pc + 2, :],
                                     start=(i == 0), stop=(i == KB // 2 - 1),
                                     perf_mode=PM.DoubleRow,
                                     skip_group_check=True)

            zb16 = actp.tile([128, HEADS], f32)

            def tail(hh):
                rbc = stg.tile([128, R], f32, tag="rbc")
                nc.vector.reciprocal(rbc[:], cds[hh][:, 0:R])
                zr = stg.tile([128, 1], f32, tag="zr")
                nc.vector.scalar_tensor_tensor(
                    junk[:], cts[hh][:, 0:R], 0.0, rbc[:],
                    ALU.bypass, ALU.mult, accum_out=zr[:])
                nc.gpsimd.tensor_add(zf[:, hh:hh + 1], zr[:], zb[:, hh:hh + 1])
                # fold this head's context into the out_proj accumulation
                nc.gpsimd.tensor_scalar_mul(zb16[:, hh:hh + 1],
                                            zf[:, hh:hh + 1], 1.0 / float(N))

            # exp chunk routing: per head 16 chunks of [128,1024];
            # 9 -> Act (exp from PSUM), 7 -> DVE copy + Pool 2^x
            POOL_SETS = [{2, 4, 6, 9, 11, 13}, {1, 3, 5, 8, 10, 12, 14}]

            def score_pair(h, pc):
                pt = pts[h]
                mc0, mc1 = 2 * pc, 2 * pc + 1
                ps_sc = psc.tile([128, 1024], f32, tag="c")
                nc.tensor.matmul(
                    ps_sc[:, 0:R],
                    ktg[:, mc0 // 4, h,
                        (mc0 % 4) * 128:(mc0 % 4) * 128 + 128],
                    qt[:, h, :], start=True, stop=True,
                    skip_group_check=True)
                nc.tensor.matmul(
                    ps_sc[:, R:2 * R],
                    ktg[:, mc1 // 4, h,
                        (mc1 % 4) * 128:(mc1 % 4) * 128 + 128],
                    qt[:, h, :], start=True, stop=True,
                    skip_group_check=True)
                if pc in POOL_SETS[h % 2]:
                    st = sce.tile([128, 1024], f32, tag="st")
                    nc.vector.tensor_copy(st[:, :], ps_sc[:, 0:1024])
                    nc.gpsimd.tensor_tensor(pt[:, 2 * pc:2 * pc + 2, :],
                                            two1k[:, :], st[:, :], ALU.pow)
                else:
                    nc.scalar.activation(pt[:, 2 * pc:2 * pc + 2, :],
                                         ps_sc[:, 0:1024], AF.Exp,
                                         scale=LN2 * SCL)

            # interleave prev head's den/ctx DR groups between score chunks
            # so the exp engines never starve at head boundaries
            for h in range(HEADS):
                pt_h = ptp.tile([128, KB, R], fp8, tag="pt")
                pts[h] = pt_h
                for pc in range(8):
                    score_pair(h, pc)
                if h >= 1:
                    pass2_den(h - 1)
                for pc in range(8, 12):
                    score_pair(h, pc)
                if h >= 1:
                    pass2_ctx(h - 1)
                for pc in range(12, KB // 2):
                    score_pair(h, pc)
                if h >= 1:
                    tail(h - 1)
            pass2_den(HEADS - 1)
            pass2_ctx(HEADS - 1)
            tail(HEADS - 1)

            # ========== folded out_proj@fc: out = (sum zf/N) @ Wfold ==========
            ps_fc = psc.tile([128, 1024], f32, tag="c")
            for h in range(HEADS):
                nc.tensor.matmul(ps_fc[0:1, 0:2], zb16[:, h:h + 1],
                                 wft[:, WF_FOLD + 2 * h:WF_FOLD + 2 * h + 2],
                                 start=(h == 0), stop=(h == HEADS - 1),
                                 skip_group_check=True)
            ores = stg.tile([1, 2], f32, tag="ores")
            nc.vector.tensor_add(ores[:], ps_fc[0:1, 0:2],
                                 wft[0:1, WF_FCB:WF_FCB + 2])
            nc.sync.dma_start(out_d[:, :], ores[:])

    nc.compile()
    return nc


def _pack_inputs(inputs):
    """Pack full inputs into per-core shards + replicated weight blocks."""
    fp8 = ml_dtypes.float8_e4m3
    bf16 = ml_dtypes.bfloat16

    adj = np.ascontiguousarray(inputs["adj_matrix"], dtype=np.float32)
    x = np.ascontiguousarray(inputs["node_features"], dtype=np.float32)
    W1 = np.asarray(inputs["W1"], np.float32)
    b1 = np.asarray(inputs["b1"], np.float32)
    W2 = np.asarray(inputs["W2"], np.float32)
    b2 = np.asarray(inputs["b2"], np.float32)
    ipw = np.asarray(inputs["in_proj_w"], np.float32)
    ipb = np.asarray(inputs["in_proj_b"], np.float32)
    wo = np.asarray(inputs["out_proj_w"], np.float32)
    bo = np.asarray(inputs["out_proj_b"], np.float32)
    fcw = np.asarray(inputs["fc_w"], np.float32)
    fcb = np.asarray(inputs["fc_b"], np.float32)

    wb = np.zeros((128, WB_COLS), np.float32)
    wb[:, WB_W1:WB_W1 + G1] = W1
    wb[:, WB_W2:WB_W2 + G2] = W2
    wb = wb.astype(bf16)

    # fp8 QKV weights: [128, c-tile, 3*G2] (q | k | v)
    w8 = np.zeros((128, ET, 3 * G2), np.float32)
    for c in range(ET):
        w8[:, c, :] = ipw[c * 128:(c + 1) * 128, :]
    w8 = w8.astype(fp8)

    wf = np.zeros((128, WF_COLS), np.float32)
    wf[:, WF_B1X4:WF_B1X4 + 4 * G1] = np.tile(b1, 4)[None, :]
    wf[:, WF_B2:WF_B2 + ET] = b2.reshape(ET, 128).T
    wf[:, WF_BQ:WF_BQ + HEADS] = ipb[0:G2].reshape(HEADS, HD).T
    wf[:, WF_BK:WF_BK + HEADS] = ipb[G2:2 * G2].reshape(HEADS, HD).T
    wf[:, WF_BV:WF_BV + HEADS] = ipb[2 * G2:3 * G2].reshape(HEADS, HD).T
    wfold = (wo.astype(np.float64) @ fcw.astype(np.float64)).astype(np.float32)
    wf[:, WF_FOLD:WF_FOLD + 2 * HEADS] = wfold.reshape(HEADS, HD, 2) \
        .transpose(1, 0, 2).reshape(HD, 2 * HEADS)
    wf[0, WF_FCB:WF_FCB + 2] = (bo.astype(np.float64) @ fcw.astype(np.float64)
                                + fcb).astype(np.float32) / NC_

    xp = np.ascontiguousarray(
        x.reshape(KB, 128, F_IN).transpose(1, 0, 2)).astype(fp8)

    reps = {"wb": wb, "w8": w8, "wf": wf, "xp": xp}
    in_maps = []
    idx = np.arange(R)
    for r in range(NC_):
        cols = np.ascontiguousarray(adj[:, r * R:(r + 1) * R])
        cols[r * R + idx, idx] += 1.0   # A + I, this core's diagonal block
        adjp = np.ascontiguousarray(
            cols.reshape(KB, 128, R).transpose(1, 0, 2)).astype(fp8)
        in_maps.append({"adjp": adjp, **reps})
    return in_maps


def kernel(**inputs):
    from concourse.bass_utils import run_bass_kernel_spmd

    if "nc" not in _cache:
        _cache["nc"] = _build()
    nc = _cache["nc"]

    in_maps = _pack_inputs(inputs)
    res = run_bass_kernel_spmd(nc, in_maps, core_ids=list(range(NC_)))
    out = np.zeros(2, dtype=np.float64)
    for r in range(NC_):
        out += res.results[r]["outp"].reshape(2).astype(np.float64)
    return out.astype(np.float32)
